# revision 45
# baseline (speedup 1.0000x reference)
"""Distributed GQA attention kernel for 8 TRN2 NeuronCores.

Sharding (tensor-parallel over heads): core i owns q-heads [8i, 8i+8) and
kv-head i (GQA n_rep=8, so one kv head serves all 8 local q heads). Each core:
  1. QKV projection from the full x in bf16 (weights/activations
     pre-transposed and pre-cast host-side), fp32 PSUM accumulation.
  2. RoPE on qT/kT in [d, s] layout (sin staged sign-folded).
  3. Causal attention per head in transposed-score layout [k, q]:
     exp(scale*s) with no max subtraction (scores are O(6)), the attention
     sink enters as +exp(sink) in the denominator, and denominators ride an
     extra ones-column appended to v.
  4. Local j-slice of the output projection -> partial yT, with wo_b/8
     folded into the eviction (sums to the exact bias across cores).
  5. Per-column-chunk bf16 ReduceScatter(add) over the 8 cores, overlapped
     with the next chunk's attention; the last chunk's scatter is row-split
     so only a small piece trails the final matmul. Core i gets yT rows
     [360i, 360i+360) (last chunk: a split row mapping). Host concatenates/
     transposes shards.
"""

import contextlib
import ctypes
import os
import sys

import numpy as np

sys.path.insert(0, "/opt/trn_rl_repo")

S = 1536
HID = 2880
D = 64
HL = 8          # local q heads per core
CORES = 8
SCQ = 512       # QKV moving chunk
NSC = S // SCQ
QC = 512        # attention q chunk
NQC = S // QC
KBN = S // 128  # 12 k blocks
VA = 68         # v_aug padded width (f32r moving dim must be 4-aligned)
CBF = 22        # full 128-row contraction blocks (2880 = 22*128 + 64)
MBN = 23        # wo output row blocks (22 full + one 64)
JBN = 4         # 512 local j rows = 4 blocks

_EXEC_TIME_NS = [None]


def _install_hooks():
    import types

    import antenv

    try:
        from antenv import axon_hooks
    except ImportError:
        axon_hooks = types.ModuleType("antenv.axon_hooks")
        _holder = {"hook": None}
        axon_hooks.set_axon_ntff_profile_hook = lambda h: _holder.update(hook=h)
        axon_hooks.get_axon_ntff_profile_hook = lambda: _holder["hook"]
        sys.modules["antenv.axon_hooks"] = axon_hooks
        antenv.axon_hooks = axon_hooks

    so_path = "/opt/axon/libaxon_pjrt.so"
    hook = None
    if os.path.exists(so_path):
        lib = ctypes.CDLL(so_path)
        if hasattr(lib, "axon_start_nrt_profile"):
            lib.axon_start_nrt_profile.argtypes = [
                ctypes.POINTER(ctypes.c_int64),
                ctypes.c_size_t,
            ]
            lib.axon_start_nrt_profile.restype = ctypes.c_int64
            lib.axon_stop_nrt_profile.argtypes = [ctypes.c_char_p]
            lib.axon_stop_nrt_profile.restype = ctypes.c_int64

            @contextlib.contextmanager
            def hook(output_dir, device_ids):
                import jax

                jax.devices()
                if device_ids:
                    ids = (ctypes.c_int64 * len(device_ids))(*device_ids)
                    rc = lib.axon_start_nrt_profile(ids, len(device_ids))
                else:
                    rc = lib.axon_start_nrt_profile(None, 0)
                if rc != 0:
                    raise RuntimeError(f"axon_start_nrt_profile rc={rc}")
                try:
                    yield
                finally:
                    n = lib.axon_stop_nrt_profile(str(output_dir).encode())
                    print(f"profile: {n} file(s) written to {output_dir}")

    axon_hooks.set_axon_ntff_profile_hook(hook)

    import concourse.bass_utils as bu

    bu.upload_artifacts = lambda tmpdir: f"local://{tmpdir}"

    if os.environ.get("BASS_LDW_OPT", "0") == "1" and not getattr(
        bu, "_ldw_patched", False
    ):
        _orig_run = bu.run_command

        def _run(cmd, *a, **k):
            cmd = [
                c.replace("--enable-ldw-opt=false", "--enable-ldw-opt=true")
                if isinstance(c, str)
                else c
                for c in cmd
            ]
            return _orig_run(cmd, *a, **k)

        bu.run_command = _run
        bu._ldw_patched = True


def build_graph():
    import concourse.mybir as mybir
    import concourse.tile as tile
    from concourse import bacc
    from concourse.masks import make_identity

    F32 = mybir.dt.float32
    BF16 = mybir.dt.bfloat16

    nc = bacc.Bacc("TRN2", target_bir_lowering=False, debug=False, num_devices=CORES)

    xT = nc.declare_dram_parameter("xT", [HID, S], BF16, isOutput=False)
    wT = nc.declare_dram_parameter("wT", [HID, 640], BF16, isOutput=False)
    bq = nc.declare_dram_parameter("bq", [64, HL], F32, isOutput=False)
    bk = nc.declare_dram_parameter("bk", [64, 1], F32, isOutput=False)
    bv = nc.declare_dram_parameter("bv", [64, 1], F32, isOutput=False)
    cosT = nc.declare_dram_parameter("cosT", [64, S], BF16, isOutput=False)
    sinTs = nc.declare_dram_parameter("sinTs", [64, S], BF16, isOutput=False)
    woT = nc.declare_dram_parameter("woT", [512, HID], BF16, isOutput=False)
    wob8 = nc.declare_dram_parameter("wob8", [128, MBN], F32, isOutput=False)
    esink = nc.declare_dram_parameter("esink", [128, HL], F32, isOutput=False)

    # qc processing order [2, 1, 0]: biggest-attention chunk first so the
    # serial ReduceScatter chain starts early; smallest chunk last.
    QC_ORDER = [2, 1, 0]
    # Row-split RS pieces per chunk: (mb_after_which_to_trigger, r0, r1).
    # Piece p covers yT rows [r0, r1); each core receives (r1-r0)/8 rows.
    # Single RS for the chunks whose transfer overlaps later compute; only the
    # last-processed chunk (qc=0) is split so less RS trails the final matmul.
    RS_PIECES = {
        2: [(22, 0, HID)],
        1: [(22, 0, HID)],
        0: [(7, 0, 1024), (22, 1024, HID)],
    }
    yT_part = {}
    yT_red = {}
    for qc in range(NQC):
        yT_part[qc] = nc.dram_tensor(f"yT_part_{qc}", [HID, QC], BF16)
        for p, (_, r0, r1) in enumerate(RS_PIECES[qc]):
            yT_red[(qc, p)] = nc.dram_tensor(
                f"yT_red_{qc}_{p}", [(r1 - r0) // 8, QC], BF16
            )
    # out row ranges per piece: piece (qc, p) lands at rows r0//8..r1//8 of
    # this core's 360-row shard; bf16 (host casts to f32).
    out = nc.declare_dram_parameter("out", [360, S], BF16, isOutput=True)

    Exp = mybir.ActivationFunctionType.Exp

    # QKV contraction pieces: groups of 128-row c-blocks (22 full + one 64-row)
    PIECES = [(0, 4), (4, 4), (8, 4), (12, 4), (16, 4), (20, 2)]

    with tile.TileContext(nc) as tc:
        with contextlib.ExitStack() as stack:
            consts = stack.enter_context(tc.tile_pool(name="consts", bufs=1))
            qkvout = stack.enter_context(tc.tile_pool(name="qkvout", bufs=1))
            small = stack.enter_context(tc.tile_pool(name="small", bufs=6))
            ytp = stack.enter_context(tc.tile_pool(name="ytp", bufs=4))

            bqt = consts.tile([64, HL], F32, tag="bq")
            bkt = consts.tile([64, 1], F32, tag="bk")
            bvt = consts.tile([64, 1], F32, tag="bv")
            cost = consts.tile([64, S], BF16, tag="cos")
            sint = consts.tile([64, S], BF16, tag="sin")
            wob8t = consts.tile([128, MBN], F32, tag="wob8")
            esk = consts.tile([128, HL], F32, tag="esk")
            ident_f = consts.tile([128, 128], F32, tag="ident_f")
            ident = consts.tile([128, 128], BF16, tag="ident")
            ones = consts.tile([128, 1], F32, tag="ones")
            # const DMAs are issued inside the QKV block, after the first
            # weight/x pieces, so the PE's first matmul data arrives first
            const_dmas = [(bqt, bq), (bkt, bk), (bvt, bv), (cost, cosT),
                          (sint, sinTs), (wob8t, wob8), (esk, esink)]
            make_identity(nc, ident_f[:, :])
            nc.vector.tensor_copy(ident[:, :], ident_f[:, :])
            nc.vector.memset(ones[:, :], 1.0)
            tri = consts.tile([128, 128], BF16, tag="tri")
            nc.vector.memset(tri[:, :], 1.0)
            nc.gpsimd.affine_select(
                out=tri[:, :], in_=tri[:, :],
                compare_op=mybir.AluOpType.is_ge,
                fill=0.0, base=0, pattern=[[1, 128]], channel_multiplier=-1,
            )

            qq = qkvout.tile([64, HL * S], BF16, tag="qq")
            kh = qkvout.tile([64, S], BF16, tag="kh")
            vT = qkvout.tile([64, S], BF16, tag="vT")
            vaug = qkvout.tile([128, KBN * VA], BF16, tag="vaug")

            # ---------------- QKV projection (piece-streamed) ----------------
            # Loop pieces outer / nb inner with 5 live PSUM banks so the first
            # matmul only needs the first weight+x piece DMAs, starting the PE
            # ~25us earlier. Weight and first-chunk x DMAs are interleaved so
            # earliest-needed data arrives first.
            with (
                tc.tile_pool(name="wtp", bufs=1) as wtp,
                tc.tile_pool(name="xcp", bufs=3) as xcp,
                tc.tile_pool(name="qkps", bufs=4, space="PSUM") as qkps,
                tc.tile_pool(name="rtmp", bufs=3) as rtmp,
            ):
                wts = []
                xps0 = []

                def dma_wt(pc):
                    cb0, ncb = PIECES[pc]
                    wt_pc = wtp.tile([128, ncb * 640], BF16, tag=f"wt{pc}",
                                     name=f"wt{pc}")
                    nc.sync.dma_start(
                        out=wt_pc[:, :].rearrange("p (cb n) -> p cb n", cb=ncb),
                        in_=wT[cb0 * 128 : (cb0 + ncb) * 128, :].rearrange(
                            "(cb p) n -> p cb n", p=128
                        ),
                    )
                    wts.append(wt_pc)

                def dma_xp(pc, sc):
                    cb0, ncb = PIECES[pc]
                    c0 = sc * SCQ
                    xp = xcp.tile([128, ncb * SCQ], BF16, tag=f"xp{pc}",
                                  name=f"xp{pc}_{sc}")
                    nc.sync.dma_start(
                        out=xp[:, :].rearrange("p (cb s) -> p cb s", cb=ncb),
                        in_=xT[cb0 * 128 : (cb0 + ncb) * 128, c0 : c0 + SCQ].rearrange(
                            "(cb p) s -> p cb s", p=128
                        ),
                    )
                    return xp

                for pc in range(len(PIECES)):
                    dma_wt(pc)
                    xps0.append(dma_xp(pc, 0))
                    if pc == 0:
                        for t, src_ in const_dmas[:3]:
                            nc.sync.dma_start(out=t[:, :], in_=src_[:, :])
                    elif pc == 1:
                        for t, src_ in const_dmas[3:]:
                            nc.sync.dma_start(out=t[:, :], in_=src_[:, :])
                wt2 = wtp.tile([64, 640], BF16, tag="wtail")
                nc.sync.dma_start(out=wt2[:, :], in_=wT[CBF * 128 : HID, :])

                for sc in range(NSC):
                    c0 = sc * SCQ
                    if sc == 0:
                        xps = xps0
                    else:
                        xps = [dma_xp(pc, sc) for pc in range(len(PIECES))]
                    xc2 = xcp.tile([64, SCQ], BF16, tag="xc2", name=f"xc2_{sc}")
                    nc.sync.dma_start(
                        out=xc2[:, :], in_=xT[CBF * 128 : HID, c0 : c0 + SCQ]
                    )

                    for nb in range(5):
                        p = qkps.tile([128, SCQ], F32, tag="qkv", name=f"qk{nb}_{sc}")
                        for pc, (cb0, ncb) in enumerate(PIECES):
                            for i in range(ncb):
                                nc.tensor.matmul(
                                    p[:, :],
                                    wts[pc][:, i * 640 + nb * 128 : i * 640 + (nb + 1) * 128],
                                    xps[pc][:, i * SCQ : (i + 1) * SCQ],
                                    start=(pc == 0 and i == 0),
                                    stop=False,
                                )
                        nc.tensor.matmul(
                            p[:, :],
                            wt2[:, nb * 128 : (nb + 1) * 128],
                            xc2[:, :],
                            start=False,
                            stop=True,
                        )
                        if nb < 4:
                            for half in range(2):
                                h = 2 * nb + half
                                hb = 64 * half
                                qb = rtmp.tile([64, SCQ], BF16, tag="qb",
                                               name=f"qb_{sc}_{nb}_{half}")
                                nc.vector.tensor_scalar_add(
                                    qb[:, :], p[hb : hb + 64, :], bqt[:, h : h + 1]
                                )
                                rot = rtmp.tile([64, SCQ], BF16, tag="rot",
                                                name=f"rot_{sc}_{nb}_{half}")
                                nc.scalar.copy(rot[0:32, :], qb[32:64, :])
                                nc.scalar.copy(rot[32:64, :], qb[0:32, :])
                                nc.vector.tensor_mul(
                                    qb[:, :], qb[:, :], cost[:, c0 : c0 + SCQ]
                                )
                                nc.vector.tensor_mul(
                                    rot[:, :], rot[:, :], sint[:, c0 : c0 + SCQ]
                                )
                                nc.vector.tensor_add(
                                    qq[:, h * S + c0 : h * S + c0 + SCQ],
                                    qb[:, :],
                                    rot[:, :],
                                )
                        else:
                            kb_ = rtmp.tile([64, SCQ], BF16, tag="qb",
                                            name=f"kb_{sc}")
                            nc.vector.tensor_scalar_add(
                                kb_[:, :], p[0:64, :], bkt[:, 0:1]
                            )
                            rot = rtmp.tile([64, SCQ], BF16, tag="rot",
                                            name=f"krot_{sc}")
                            nc.scalar.copy(rot[0:32, :], kb_[32:64, :])
                            nc.scalar.copy(rot[32:64, :], kb_[0:32, :])
                            nc.vector.tensor_mul(
                                kb_[:, :], kb_[:, :], cost[:, c0 : c0 + SCQ]
                            )
                            nc.vector.tensor_mul(
                                rot[:, :], rot[:, :], sint[:, c0 : c0 + SCQ]
                            )
                            nc.vector.tensor_add(
                                kh[:, c0 : c0 + SCQ], kb_[:, :], rot[:, :]
                            )
                            nc.vector.tensor_scalar_add(
                                vT[:, c0 : c0 + SCQ], p[64:128, :], bvt[:, 0:1]
                            )

            # ---------------- v transpose + ones column ----------------
            with tc.tile_pool(name="vtp", bufs=2, space="PSUM") as vtp:
                for kb in range(KBN):
                    pv = vtp.tile([128, D], BF16, tag="pv", name=f"pv{kb}")
                    nc.tensor.transpose(
                        pv[:, :], vT[:, kb * 128 : (kb + 1) * 128], ident[0:64, 0:64]
                    )
                    nc.vector.tensor_copy(vaug[:, kb * VA : kb * VA + 64], pv[:, :])
                    for oc in range(64, VA):
                        nc.vector.tensor_copy(
                            vaug[:, kb * VA + oc : kb * VA + oc + 1], ones[:, :]
                        )

            # ---------------- attention + wo + chunked ReduceScatter ----------------
            with (
                tc.tile_pool(name="oTp", bufs=1) as oTp,
                tc.tile_pool(name="woTp", bufs=1) as woTp,
                tc.tile_pool(name="esp", bufs=14) as esp,
                tc.tile_pool(name="scps", bufs=2, space="PSUM") as scps,
                tc.tile_pool(name="pops", bufs=1, space="PSUM") as pops,
                tc.tile_pool(name="wops", bufs=2, space="PSUM") as wops,
            ):
                oTt = oTp.tile([128, JBN * S], BF16, tag="oT")
                woTt = woTp.tile([128, JBN * HID], BF16, tag="woT")
                nc.sync.dma_start(
                    out=woTt[:, :].rearrange("p (jb m) -> p jb m", jb=JBN),
                    in_=woT[:, :].rearrange("(jb p) m -> p jb m", p=128),
                )

                def emit_wo_block(qc, mb, piece_state):
                    # One wo output-row block: 4 accumulating matmuls over jb,
                    # bias-add eviction (vector), DMA to yT_part, and the RS
                    # trigger + out-DMA when this mb completes a row piece.
                    q0 = qc * QC
                    rows = 128 if mb < CBF else 64
                    pw = wops.tile([128, QC], F32, tag="wo",
                                   name=f"pw_{qc}_{mb}")
                    for jb in range(JBN):
                        nc.tensor.matmul(
                            pw[0:rows, :],
                            woTt[:, jb * HID + mb * 128 : jb * HID + mb * 128 + rows],
                            oTt[:, jb * S + q0 : jb * S + q0 + QC],
                            start=(jb == 0),
                            stop=(jb == JBN - 1),
                        )
                    yt = ytp.tile([128, QC], BF16, tag="ytb",
                                  name=f"yt_{qc}_{mb}")
                    if mb % 2 == 0:
                        nc.vector.tensor_scalar_add(
                            yt[0:rows, :], pw[0:rows, :],
                            wob8t[0:rows, mb : mb + 1],
                        )
                    else:
                        nc.scalar.activation(
                            yt[0:rows, :], pw[0:rows, :],
                            mybir.ActivationFunctionType.Identity,
                            bias=wob8t[0:rows, mb : mb + 1],
                        )
                    nc.sync.dma_start(
                        out=yT_part[qc][mb * 128 : mb * 128 + rows, :],
                        in_=yt[0:rows, :],
                    )
                    piece = piece_state[0]
                    if (piece < len(RS_PIECES[qc])
                            and mb == RS_PIECES[qc][piece][0]):
                        _, r0, r1 = RS_PIECES[qc][piece]
                        nc.gpsimd.collective_compute(
                            "ReduceScatter",
                            mybir.AluOpType.add,
                            replica_groups=[list(range(CORES))],
                            ins=[yT_part[qc][r0:r1, :].opt()],
                            outs=[yT_red[(qc, piece)].ap().opt()],
                        )
                        o0 = r0 // CORES
                        nrows = (r1 - r0) // CORES
                        # issue from gpsimd: this DMA waits on the collective,
                        # and must not block the sync queue's evictions
                        nc.gpsimd.dma_start(
                            out=out[o0 : o0 + nrows, q0 : q0 + QC],
                            in_=yT_red[(qc, piece)].ap(),
                        )
                        piece_state[0] += 1

                def emit_score_pair(qc, h, pk):
                    # Two k-blocks into one 2-bank PSUM tile, one exp over
                    # [128, 1024] (halves the scalar engine's fixed cost).
                    # Diagonal k-blocks compute full width; the sub-diagonal
                    # garbage is never read (AV skips s < j) and only the
                    # diagonal 128-block needs the triangular mask.
                    q0 = qc * QC
                    qb0 = q0 // 128
                    ps_s = scps.tile([128, 2 * QC], F32, tag="scores",
                                     name=f"ps_{qc}_{h}_{pk}")
                    for half in range(2):
                        kb = 2 * pk + half
                        j = kb - qb0
                        w0 = 128 * j if j > 0 else 0
                        nc.tensor.matmul(
                            ps_s[:, half * QC + w0 : (half + 1) * QC],
                            kh[:, kb * 128 : (kb + 1) * 128],
                            qq[:, h * S + q0 + w0 : h * S + q0 + QC],
                            start=True,
                            stop=True,
                        )
                    es = esp.tile([128, 2 * QC], BF16, tag="es",
                                  name=f"es_{qc}_{h}_{pk}")
                    nc.scalar.activation(es[:, :], ps_s[:, :], Exp, scale=0.125)
                    for half in range(2):
                        kb = 2 * pk + half
                        j = kb - qb0
                        if j >= 0:
                            dc = half * QC + 128 * j
                            # gpsimd is collective-free until the first RS
                            # trigger (mid-attn of the second chunk), so the
                            # first chunk's even masks can run there
                            if qc == QC_ORDER[0]:
                                nc.gpsimd.affine_select(
                                    out=es[:, dc : dc + 128],
                                    in_=es[:, dc : dc + 128],
                                    compare_op=mybir.AluOpType.is_ge,
                                    fill=0.0,
                                    base=0,
                                    pattern=[[1, 128]],
                                    channel_multiplier=-1,
                                )
                            else:
                                nc.vector.tensor_mul(
                                    es[:, dc : dc + 128],
                                    es[:, dc : dc + 128],
                                    tri[:, :],
                                )
                    return es

                def emit_av_wave(qc, h, es_pairs, wave):
                    # AV accumulation for q sub-blocks (2*wave, 2*wave+1) plus
                    # the vector normalize chain; the pt transposes are
                    # deferred (emit_pts) so the PE isn't parked behind the
                    # vector chain.
                    qb0 = qc * QC // 128
                    po = {}
                    for s in (2 * wave, 2 * wave + 1):
                        po[s] = pops.tile([128, VA], F32, tag=f"po{s % 2}",
                                          name=f"po{s}_{qc}_{h}")
                        for kb in range(qb0 + s + 1):
                            nc.tensor.matmul(
                                po[s][:, :],
                                es_pairs[kb // 2][:, (kb % 2) * QC + s * 128 :
                                                  (kb % 2) * QC + s * 128 + 128],
                                vaug[:, kb * VA : (kb + 1) * VA],
                                start=(kb == 0),
                                stop=(kb == qb0 + s),
                            )
                    o_ns = {}
                    for s in (2 * wave, 2 * wave + 1):
                        denom = small.tile([128, 1], F32, tag="denom",
                                           name=f"dn_{qc}_{h}_{s}")
                        nc.vector.tensor_add(
                            denom[:, :], po[s][:, D : D + 1], esk[:, h : h + 1]
                        )
                        recip = small.tile([128, 1], F32, tag="recip",
                                           name=f"rc_{qc}_{h}_{s}")
                        nc.vector.reciprocal(recip[:, :], denom[:, :])
                        o_n = small.tile([128, D], BF16, tag="o_n",
                                         name=f"on_{qc}_{h}_{s}")
                        nc.vector.tensor_scalar_mul(
                            o_n[:, :], po[s][:, 0:D], recip[:, :]
                        )
                        o_ns[s] = o_n
                    return o_ns

                def emit_pts(qc, h, o_ns):
                    # Transpose normalized outputs into oTt; pt tiles reuse
                    # the po banks (tag-shared) to stay inside the 8-bank
                    # PSUM budget.
                    q0 = qc * QC
                    for s, o_n in o_ns.items():
                        pt = pops.tile([64, 128], BF16, tag=f"po{s % 2}",
                                       name=f"pt_{qc}_{h}_{s}")
                        nc.tensor.transpose(pt[:, :], o_n[:, :], ident[:, :])
                        jb, ro = h // 2, (h % 2) * 64
                        dst = oTt[
                            ro : ro + 64,
                            jb * S + q0 + s * 128 : jb * S + q0 + (s + 1) * 128,
                        ]
                        if s % 2 == 0:
                            nc.vector.tensor_copy(dst, pt[:, :])
                        else:
                            nc.scalar.copy(dst, pt[:, :])

                # Software pipeline: head h's score pairs are spread between
                # head h-1's AV waves (so exps drain in the shadow of AV/wo
                # work instead of stalling the score PSUM rotation), and the
                # previous chunk's wo blocks fill remaining PE slack. The PE
                # stays continuously busy, which also keeps its DVFS p-state
                # at full clock.
                WO_PER_SLOT = 6
                pending = []

                def fill_wo(n):
                    for _ in range(n):
                        if pending:
                            pending.pop(0)()

                def slot(qc, h, prev_es):
                    # Emit one pipeline slot: all score pairs for head h, then
                    # the AV waves + normalize + transposes of head h-1
                    # (prev_es), then wo filler. h == HL drains only.
                    npairs = (qc * QC // 128 + 4) // 2
                    es_pairs = []
                    if h < HL:
                        for pk in range(npairs):
                            es_pairs.append(emit_score_pair(qc, h, pk))
                    if prev_es:
                        o_a = emit_av_wave(qc, h - 1, prev_es, 0)
                        emit_pts(qc, h - 1, o_a)
                        o_b = emit_av_wave(qc, h - 1, prev_es, 1)
                        emit_pts(qc, h - 1, o_b)
                    fill_wo(WO_PER_SLOT)
                    return es_pairs

                for qc in QC_ORDER:
                    prev = None
                    for h in range(HL + 1):
                        prev = slot(qc, h, prev)
                    while pending:
                        pending.pop(0)()
                    ps_ = [0]
                    if qc != 0:
                        pending = [
                            (lambda qc=qc, mb=mb, ps_=ps_:
                             emit_wo_block(qc, mb, ps_))
                            for mb in range(MBN)
                        ]
                    else:
                        for mb in range(MBN):
                            emit_wo_block(qc, mb, ps_)

    nc.finalize()
    return nc


def make_in_maps(x, rope_cache, wq_w, wq_b, wk_w, wk_b, wv_w, wv_b, wo_w, wo_b, sinks):
    import ml_dtypes

    BF = ml_dtypes.bfloat16
    xT = np.ascontiguousarray(x[0].T).astype(BF)  # [2880, 1536]
    cosT = np.ascontiguousarray(rope_cache[:, :D].T, dtype=np.float32)
    sinT = np.ascontiguousarray(rope_cache[:, D:].T, dtype=np.float32)
    sinTs = sinT.copy()
    sinTs[: D // 2] *= -1.0
    cosT = cosT.astype(BF)
    sinTs = sinTs.astype(BF)

    in_maps = []
    for i in range(CORES):
        wq = wq_w[512 * i : 512 * (i + 1)]
        wk = wk_w[64 * i : 64 * (i + 1)]
        wv = wv_w[64 * i : 64 * (i + 1)]
        wT = np.ascontiguousarray(np.concatenate([wq, wk, wv], axis=0).T).astype(BF)
        bq = np.ascontiguousarray(
            wq_b[512 * i : 512 * (i + 1)].reshape(HL, 64).T, np.float32
        )
        bk = wk_b[64 * i : 64 * (i + 1)].reshape(64, 1).astype(np.float32)
        bv = wv_b[64 * i : 64 * (i + 1)].reshape(64, 1).astype(np.float32)
        woT = np.ascontiguousarray(wo_w[:, 512 * i : 512 * (i + 1)].T).astype(BF)
        wob8 = np.zeros((128, MBN), np.float32)
        for mb in range(MBN):
            piece = wo_b[128 * mb : 128 * (mb + 1)] / 8.0
            wob8[: len(piece), mb] = piece
        es = np.exp(sinks[HL * i : HL * (i + 1)]).astype(np.float32)
        esink = np.repeat(es[None, :], 128, axis=0).astype(np.float32)
        in_maps.append(
            {
                "xT": xT,
                "wT": wT,
                "bq": bq,
                "bk": bk,
                "bv": bv,
                "cosT": cosT,
                "sinTs": sinTs,
                "woT": woT,
                "wob8": np.ascontiguousarray(wob8),
                "esink": esink,
            }
        )
    return in_maps


_CACHE = {}


def kernel(**inputs):
    _install_hooks()
    from concourse import bass_utils

    trace = bool(int(os.environ.get("BASS_KERNEL_TRACE", "0")))
    if "nc" not in _CACHE:
        _CACHE["nc"] = build_graph()
    nc = _CACHE["nc"]

    in_maps = make_in_maps(**{k: np.asarray(v) for k, v in inputs.items()})
    res = bass_utils.run_bass_kernel_spmd(
        nc, in_maps, core_ids=list(range(CORES)), trace=trace
    )
    _EXEC_TIME_NS[0] = res.exec_time_ns

    # Assemble: piece (qc, p) covering yT rows [r0, r1) sits at rows
    # [r0/8, r1/8) of core i's 360-row shard and holds global yT rows
    # [r0 + i*(r1-r0)/8, ...) for q columns [qc*QC, qc*QC+QC).
    RS_PIECES = {2: [(0, HID)], 1: [(0, HID)], 0: [(0, 1024), (1024, HID)]}
    y = np.empty((S, HID), np.float32)
    for i in range(CORES):
        o = res.results[i]["out"].astype(np.float32)
        for qc in range(NQC):
            for r0, r1 in RS_PIECES[qc]:
                rows = (r1 - r0) // CORES
                o0 = r0 // CORES
                y[qc * QC : (qc + 1) * QC,
                  r0 + i * rows : r0 + (i + 1) * rows] = (
                    o[o0 : o0 + rows, qc * QC : (qc + 1) * QC].T
                )
    return y.reshape(1, S, HID)


def last_exec_time_ns():
    return _EXEC_TIME_NS[0]



# revision 46
# speedup vs baseline: 1.0219x; 1.0219x over previous
"""Distributed GQA attention kernel for 8 TRN2 NeuronCores.

Sharding (tensor-parallel over heads): core i owns q-heads [8i, 8i+8) and
kv-head i (GQA n_rep=8, so one kv head serves all 8 local q heads). Each core:
  1. QKV projection from the full x in bf16 (weights/activations
     pre-transposed and pre-cast host-side), fp32 PSUM accumulation.
  2. RoPE on qT/kT in [d, s] layout (sin staged sign-folded).
  3. Causal attention per head in transposed-score layout [k, q]:
     exp(scale*s) with no max subtraction (scores are O(6)), the attention
     sink enters as +exp(sink) in the denominator, and denominators ride an
     extra ones-column appended to v.
  4. Local j-slice of the output projection -> partial yT, with wo_b/8
     folded into the eviction (sums to the exact bias across cores).
  5. Per-column-chunk bf16 ReduceScatter(add) over the 8 cores, overlapped
     with the next chunk's attention; the last chunk's scatter is row-split
     so only a small piece trails the final matmul. Core i gets yT rows
     [360i, 360i+360) (last chunk: a split row mapping). Host concatenates/
     transposes shards.
"""

import contextlib
import ctypes
import os
import sys

import numpy as np

sys.path.insert(0, "/opt/trn_rl_repo")

S = 1536
HID = 2880
D = 64
HL = 8          # local q heads per core
CORES = 8
SCQ = 512       # QKV moving chunk
NSC = S // SCQ
QC = 512        # attention q chunk
NQC = S // QC
KBN = S // 128  # 12 k blocks
VA = 68         # v_aug padded width (f32r moving dim must be 4-aligned)
CBF = 22        # full 128-row contraction blocks (2880 = 22*128 + 64)
MBN = 23        # wo output row blocks (22 full + one 64)
JBN = 4         # 512 local j rows = 4 blocks

_EXEC_TIME_NS = [None]


def _install_hooks():
    import types

    import antenv

    try:
        from antenv import axon_hooks
    except ImportError:
        axon_hooks = types.ModuleType("antenv.axon_hooks")
        _holder = {"hook": None}
        axon_hooks.set_axon_ntff_profile_hook = lambda h: _holder.update(hook=h)
        axon_hooks.get_axon_ntff_profile_hook = lambda: _holder["hook"]
        sys.modules["antenv.axon_hooks"] = axon_hooks
        antenv.axon_hooks = axon_hooks

    so_path = "/opt/axon/libaxon_pjrt.so"
    hook = None
    if os.path.exists(so_path):
        lib = ctypes.CDLL(so_path)
        if hasattr(lib, "axon_start_nrt_profile"):
            lib.axon_start_nrt_profile.argtypes = [
                ctypes.POINTER(ctypes.c_int64),
                ctypes.c_size_t,
            ]
            lib.axon_start_nrt_profile.restype = ctypes.c_int64
            lib.axon_stop_nrt_profile.argtypes = [ctypes.c_char_p]
            lib.axon_stop_nrt_profile.restype = ctypes.c_int64

            @contextlib.contextmanager
            def hook(output_dir, device_ids):
                import jax

                jax.devices()
                if device_ids:
                    ids = (ctypes.c_int64 * len(device_ids))(*device_ids)
                    rc = lib.axon_start_nrt_profile(ids, len(device_ids))
                else:
                    rc = lib.axon_start_nrt_profile(None, 0)
                if rc != 0:
                    raise RuntimeError(f"axon_start_nrt_profile rc={rc}")
                try:
                    yield
                finally:
                    n = lib.axon_stop_nrt_profile(str(output_dir).encode())
                    print(f"profile: {n} file(s) written to {output_dir}")

    axon_hooks.set_axon_ntff_profile_hook(hook)

    import concourse.bass_utils as bu

    bu.upload_artifacts = lambda tmpdir: f"local://{tmpdir}"

    if os.environ.get("BASS_LDW_OPT", "0") == "1" and not getattr(
        bu, "_ldw_patched", False
    ):
        _orig_run = bu.run_command

        def _run(cmd, *a, **k):
            cmd = [
                c.replace("--enable-ldw-opt=false", "--enable-ldw-opt=true")
                if isinstance(c, str)
                else c
                for c in cmd
            ]
            return _orig_run(cmd, *a, **k)

        bu.run_command = _run
        bu._ldw_patched = True


def build_graph():
    import concourse.mybir as mybir
    import concourse.tile as tile
    from concourse import bacc
    from concourse.masks import make_identity

    F32 = mybir.dt.float32
    BF16 = mybir.dt.bfloat16

    nc = bacc.Bacc("TRN2", target_bir_lowering=False, debug=False, num_devices=CORES)

    xT = nc.declare_dram_parameter("xT", [HID, S], BF16, isOutput=False)
    wT = nc.declare_dram_parameter("wT", [HID, 640], BF16, isOutput=False)
    bq = nc.declare_dram_parameter("bq", [64, HL], F32, isOutput=False)
    bk = nc.declare_dram_parameter("bk", [64, 1], F32, isOutput=False)
    bv = nc.declare_dram_parameter("bv", [64, 1], F32, isOutput=False)
    cosT = nc.declare_dram_parameter("cosT", [64, S], BF16, isOutput=False)
    sinTs = nc.declare_dram_parameter("sinTs", [64, S], BF16, isOutput=False)
    woT = nc.declare_dram_parameter("woT", [512, HID], BF16, isOutput=False)
    wob8 = nc.declare_dram_parameter("wob8", [128, MBN], F32, isOutput=False)
    esink = nc.declare_dram_parameter("esink", [128, HL], F32, isOutput=False)

    # qc processing order [2, 1, 0]: biggest-attention chunk first so the
    # serial ReduceScatter chain starts early; smallest chunk last.
    QC_ORDER = [2, 1, 0]
    # Row-split RS pieces per chunk: (mb_after_which_to_trigger, r0, r1).
    # Piece p covers yT rows [r0, r1); each core receives (r1-r0)/8 rows.
    # Single RS for the chunks whose transfer overlaps later compute; only the
    # last-processed chunk (qc=0) is split so less RS trails the final matmul.
    RS_PIECES = {
        2: [(22, 0, HID)],
        1: [(22, 0, HID)],
        0: [(7, 0, 1024), (22, 1024, HID)],
    }
    yT_part = {}
    yT_red = {}
    for qc in range(NQC):
        yT_part[qc] = nc.dram_tensor(f"yT_part_{qc}", [HID, QC], BF16)
        for p, (_, r0, r1) in enumerate(RS_PIECES[qc]):
            yT_red[(qc, p)] = nc.dram_tensor(
                f"yT_red_{qc}_{p}", [(r1 - r0) // 8, QC], BF16
            )
    # out row ranges per piece: piece (qc, p) lands at rows r0//8..r1//8 of
    # this core's 360-row shard; bf16 (host casts to f32).
    out = nc.declare_dram_parameter("out", [360, S], BF16, isOutput=True)

    Exp = mybir.ActivationFunctionType.Exp

    # QKV contraction pieces: groups of 128-row c-blocks (22 full + one 64-row)
    PIECES = [(0, 4), (4, 4), (8, 4), (12, 4), (16, 4), (20, 2)]

    with tile.TileContext(nc) as tc:
        with contextlib.ExitStack() as stack:
            consts = stack.enter_context(tc.tile_pool(name="consts", bufs=1))
            qkvout = stack.enter_context(tc.tile_pool(name="qkvout", bufs=1))
            small = stack.enter_context(tc.tile_pool(name="small", bufs=6))
            ytp = stack.enter_context(tc.tile_pool(name="ytp", bufs=4))

            bqt = consts.tile([64, HL], F32, tag="bq")
            bkt = consts.tile([64, 1], F32, tag="bk")
            bvt = consts.tile([64, 1], F32, tag="bv")
            cost = consts.tile([64, S], BF16, tag="cos")
            sint = consts.tile([64, S], BF16, tag="sin")
            wob8t = consts.tile([128, MBN], F32, tag="wob8")
            esk = consts.tile([128, HL], F32, tag="esk")
            ident_f = consts.tile([128, 128], F32, tag="ident_f")
            ident = consts.tile([128, 128], BF16, tag="ident")
            ones = consts.tile([128, 1], F32, tag="ones")
            # const DMAs are issued inside the QKV block, after the first
            # weight/x pieces, so the PE's first matmul data arrives first
            const_dmas = [(bqt, bq), (bkt, bk), (bvt, bv), (cost, cosT),
                          (sint, sinTs), (wob8t, wob8), (esk, esink)]
            make_identity(nc, ident_f[:, :])
            nc.vector.tensor_copy(ident[:, :], ident_f[:, :])
            nc.vector.memset(ones[:, :], 1.0)
            tri = consts.tile([128, 128], BF16, tag="tri")
            nc.vector.memset(tri[:, :], 1.0)
            nc.gpsimd.affine_select(
                out=tri[:, :], in_=tri[:, :],
                compare_op=mybir.AluOpType.is_ge,
                fill=0.0, base=0, pattern=[[1, 128]], channel_multiplier=-1,
            )

            qq = qkvout.tile([64, HL * S], BF16, tag="qq")
            kh = qkvout.tile([64, S], BF16, tag="kh")
            vT = qkvout.tile([64, S], BF16, tag="vT")
            vaug = qkvout.tile([128, KBN * VA], BF16, tag="vaug")

            # ---------------- QKV projection (piece-streamed) ----------------
            # Loop pieces outer / nb inner with 5 live PSUM banks so the first
            # matmul only needs the first weight+x piece DMAs, starting the PE
            # ~25us earlier. Weight and first-chunk x DMAs are interleaved so
            # earliest-needed data arrives first.
            with (
                tc.tile_pool(name="wtp", bufs=1) as wtp,
                tc.tile_pool(name="xcp", bufs=3) as xcp,
                tc.tile_pool(name="qkps", bufs=4, space="PSUM") as qkps,
                tc.tile_pool(name="rtmp", bufs=3) as rtmp,
            ):
                wts = []
                xps0 = []

                def dma_wt(pc):
                    cb0, ncb = PIECES[pc]
                    wt_pc = wtp.tile([128, ncb * 640], BF16, tag=f"wt{pc}",
                                     name=f"wt{pc}")
                    nc.sync.dma_start(
                        out=wt_pc[:, :].rearrange("p (cb n) -> p cb n", cb=ncb),
                        in_=wT[cb0 * 128 : (cb0 + ncb) * 128, :].rearrange(
                            "(cb p) n -> p cb n", p=128
                        ),
                    )
                    wts.append(wt_pc)

                def dma_xp(pc, sc):
                    cb0, ncb = PIECES[pc]
                    c0 = sc * SCQ
                    xp = xcp.tile([128, ncb * SCQ], BF16, tag=f"xp{pc}",
                                  name=f"xp{pc}_{sc}")
                    nc.sync.dma_start(
                        out=xp[:, :].rearrange("p (cb s) -> p cb s", cb=ncb),
                        in_=xT[cb0 * 128 : (cb0 + ncb) * 128, c0 : c0 + SCQ].rearrange(
                            "(cb p) s -> p cb s", p=128
                        ),
                    )
                    return xp

                for pc in range(len(PIECES)):
                    dma_wt(pc)
                    xps0.append(dma_xp(pc, 0))
                    if pc == 0:
                        for t, src_ in const_dmas[:3]:
                            nc.sync.dma_start(out=t[:, :], in_=src_[:, :])
                    elif pc == 1:
                        for t, src_ in const_dmas[3:]:
                            nc.sync.dma_start(out=t[:, :], in_=src_[:, :])
                wt2 = wtp.tile([64, 640], BF16, tag="wtail")
                nc.sync.dma_start(out=wt2[:, :], in_=wT[CBF * 128 : HID, :])

                for sc in range(NSC):
                    c0 = sc * SCQ
                    if sc == 0:
                        xps = xps0
                    else:
                        xps = [dma_xp(pc, sc) for pc in range(len(PIECES))]
                    xc2 = xcp.tile([64, SCQ], BF16, tag="xc2", name=f"xc2_{sc}")
                    nc.sync.dma_start(
                        out=xc2[:, :], in_=xT[CBF * 128 : HID, c0 : c0 + SCQ]
                    )

                    for nb in range(5):
                        p = qkps.tile([128, SCQ], F32, tag="qkv", name=f"qk{nb}_{sc}")
                        for pc, (cb0, ncb) in enumerate(PIECES):
                            for i in range(ncb):
                                nc.tensor.matmul(
                                    p[:, :],
                                    wts[pc][:, i * 640 + nb * 128 : i * 640 + (nb + 1) * 128],
                                    xps[pc][:, i * SCQ : (i + 1) * SCQ],
                                    start=(pc == 0 and i == 0),
                                    stop=False,
                                )
                        nc.tensor.matmul(
                            p[:, :],
                            wt2[:, nb * 128 : (nb + 1) * 128],
                            xc2[:, :],
                            start=False,
                            stop=True,
                        )
                        if nb < 4:
                            for half in range(2):
                                h = 2 * nb + half
                                hb = 64 * half
                                qb = rtmp.tile([64, SCQ], BF16, tag="qb",
                                               name=f"qb_{sc}_{nb}_{half}")
                                nc.vector.tensor_scalar_add(
                                    qb[:, :], p[hb : hb + 64, :], bqt[:, h : h + 1]
                                )
                                rot = rtmp.tile([64, SCQ], BF16, tag="rot",
                                                name=f"rot_{sc}_{nb}_{half}")
                                nc.scalar.copy(rot[0:32, :], qb[32:64, :])
                                nc.scalar.copy(rot[32:64, :], qb[0:32, :])
                                nc.vector.tensor_mul(
                                    qb[:, :], qb[:, :], cost[:, c0 : c0 + SCQ]
                                )
                                nc.vector.tensor_mul(
                                    rot[:, :], rot[:, :], sint[:, c0 : c0 + SCQ]
                                )
                                nc.vector.tensor_add(
                                    qq[:, h * S + c0 : h * S + c0 + SCQ],
                                    qb[:, :],
                                    rot[:, :],
                                )
                        else:
                            kb_ = rtmp.tile([64, SCQ], BF16, tag="qb",
                                            name=f"kb_{sc}")
                            nc.vector.tensor_scalar_add(
                                kb_[:, :], p[0:64, :], bkt[:, 0:1]
                            )
                            rot = rtmp.tile([64, SCQ], BF16, tag="rot",
                                            name=f"krot_{sc}")
                            nc.scalar.copy(rot[0:32, :], kb_[32:64, :])
                            nc.scalar.copy(rot[32:64, :], kb_[0:32, :])
                            nc.vector.tensor_mul(
                                kb_[:, :], kb_[:, :], cost[:, c0 : c0 + SCQ]
                            )
                            nc.vector.tensor_mul(
                                rot[:, :], rot[:, :], sint[:, c0 : c0 + SCQ]
                            )
                            nc.vector.tensor_add(
                                kh[:, c0 : c0 + SCQ], kb_[:, :], rot[:, :]
                            )
                            nc.vector.tensor_scalar_add(
                                vT[:, c0 : c0 + SCQ], p[64:128, :], bvt[:, 0:1]
                            )

            # ---------------- v transpose + ones column ----------------
            with tc.tile_pool(name="vtp", bufs=2, space="PSUM") as vtp:
                for kb in range(KBN):
                    pv = vtp.tile([128, D], BF16, tag="pv", name=f"pv{kb}")
                    nc.tensor.transpose(
                        pv[:, :], vT[:, kb * 128 : (kb + 1) * 128], ident[0:64, 0:64]
                    )
                    nc.vector.tensor_copy(vaug[:, kb * VA : kb * VA + 64], pv[:, :])
                    for oc in range(64, VA):
                        nc.vector.tensor_copy(
                            vaug[:, kb * VA + oc : kb * VA + oc + 1], ones[:, :]
                        )

            # ---------------- attention + wo + chunked ReduceScatter ----------------
            with (
                tc.tile_pool(name="oTp", bufs=1) as oTp,
                tc.tile_pool(name="woTp", bufs=1) as woTp,
                tc.tile_pool(name="esp", bufs=14) as esp,
                tc.tile_pool(name="scps", bufs=2, space="PSUM") as scps,
                tc.tile_pool(name="pops", bufs=1, space="PSUM") as pops,
                tc.tile_pool(name="wops", bufs=2, space="PSUM") as wops,
            ):
                oTt = oTp.tile([128, JBN * S], BF16, tag="oT")
                woTt = woTp.tile([128, JBN * HID], BF16, tag="woT")
                nc.sync.dma_start(
                    out=woTt[:, :].rearrange("p (jb m) -> p jb m", jb=JBN),
                    in_=woT[:, :].rearrange("(jb p) m -> p jb m", p=128),
                )

                def emit_wo_block(qc, mb, piece_state):
                    # One wo output-row block: 4 accumulating matmuls over jb,
                    # bias-add eviction (vector), DMA to yT_part, and the RS
                    # trigger + out-DMA when this mb completes a row piece.
                    q0 = qc * QC
                    rows = 128 if mb < CBF else 64
                    pw = wops.tile([128, QC], F32, tag="wo",
                                   name=f"pw_{qc}_{mb}")
                    for jb in range(JBN):
                        nc.tensor.matmul(
                            pw[0:rows, :],
                            woTt[:, jb * HID + mb * 128 : jb * HID + mb * 128 + rows],
                            oTt[:, jb * S + q0 : jb * S + q0 + QC],
                            start=(jb == 0),
                            stop=(jb == JBN - 1),
                        )
                    yt = ytp.tile([128, QC], BF16, tag="ytb",
                                  name=f"yt_{qc}_{mb}")
                    if mb % 2 == 0:
                        nc.vector.tensor_scalar_add(
                            yt[0:rows, :], pw[0:rows, :],
                            wob8t[0:rows, mb : mb + 1],
                        )
                    else:
                        nc.scalar.activation(
                            yt[0:rows, :], pw[0:rows, :],
                            mybir.ActivationFunctionType.Identity,
                            bias=wob8t[0:rows, mb : mb + 1],
                        )
                    nc.sync.dma_start(
                        out=yT_part[qc][mb * 128 : mb * 128 + rows, :],
                        in_=yt[0:rows, :],
                    )
                    piece = piece_state[0]
                    if (piece < len(RS_PIECES[qc])
                            and mb == RS_PIECES[qc][piece][0]):
                        _, r0, r1 = RS_PIECES[qc][piece]
                        nc.gpsimd.collective_compute(
                            "ReduceScatter",
                            mybir.AluOpType.add,
                            replica_groups=[list(range(CORES))],
                            ins=[yT_part[qc][r0:r1, :].opt()],
                            outs=[yT_red[(qc, piece)].ap().opt()],
                        )
                        o0 = r0 // CORES
                        nrows = (r1 - r0) // CORES
                        # issue from gpsimd: this DMA waits on the collective,
                        # and must not block the sync queue's evictions
                        nc.gpsimd.dma_start(
                            out=out[o0 : o0 + nrows, q0 : q0 + QC],
                            in_=yT_red[(qc, piece)].ap(),
                        )
                        piece_state[0] += 1

                def emit_score_pair(qc, h, pk):
                    # Two k-blocks into one 2-bank PSUM tile, one exp over
                    # [128, 1024] (halves the scalar engine's fixed cost).
                    # Diagonal k-blocks compute full width; the sub-diagonal
                    # garbage is never read (AV skips s < j) and only the
                    # diagonal 128-block needs the triangular mask.
                    q0 = qc * QC
                    qb0 = q0 // 128
                    ps_s = scps.tile([128, 2 * QC], F32, tag="scores",
                                     name=f"ps_{qc}_{h}_{pk}")
                    for half in range(2):
                        kb = 2 * pk + half
                        j = kb - qb0
                        w0 = 128 * j if j > 0 else 0
                        nc.tensor.matmul(
                            ps_s[:, half * QC + w0 : (half + 1) * QC],
                            kh[:, kb * 128 : (kb + 1) * 128],
                            qq[:, h * S + q0 + w0 : h * S + q0 + QC],
                            start=True,
                            stop=True,
                        )
                    es = esp.tile([128, 2 * QC], BF16, tag="es",
                                  name=f"es_{qc}_{h}_{pk}")
                    nc.scalar.activation(es[:, :], ps_s[:, :], Exp, scale=0.125)
                    for half in range(2):
                        kb = 2 * pk + half
                        j = kb - qb0
                        if j >= 0:
                            dc = half * QC + 128 * j
                            # gpsimd is collective-free until the first RS
                            # trigger (mid-attn of the second chunk), so the
                            # first chunk's even masks can run there
                            if qc == QC_ORDER[0] and kb % 2 == 0:
                                nc.gpsimd.affine_select(
                                    out=es[:, dc : dc + 128],
                                    in_=es[:, dc : dc + 128],
                                    compare_op=mybir.AluOpType.is_ge,
                                    fill=0.0,
                                    base=0,
                                    pattern=[[1, 128]],
                                    channel_multiplier=-1,
                                )
                            else:
                                nc.vector.tensor_mul(
                                    es[:, dc : dc + 128],
                                    es[:, dc : dc + 128],
                                    tri[:, :],
                                )
                    return es

                def emit_av_wave(qc, h, es_pairs, wave):
                    # AV accumulation for q sub-blocks (2*wave, 2*wave+1) plus
                    # the vector normalize chain; the pt transposes are
                    # deferred (emit_pts) so the PE isn't parked behind the
                    # vector chain.
                    qb0 = qc * QC // 128
                    po = {}
                    for s in (2 * wave, 2 * wave + 1):
                        po[s] = pops.tile([128, VA], F32, tag=f"po{s % 2}",
                                          name=f"po{s}_{qc}_{h}")
                        for kb in range(qb0 + s + 1):
                            nc.tensor.matmul(
                                po[s][:, :],
                                es_pairs[kb // 2][:, (kb % 2) * QC + s * 128 :
                                                  (kb % 2) * QC + s * 128 + 128],
                                vaug[:, kb * VA : (kb + 1) * VA],
                                start=(kb == 0),
                                stop=(kb == qb0 + s),
                            )
                    o_ns = {}
                    for s in (2 * wave, 2 * wave + 1):
                        denom = small.tile([128, 1], F32, tag="denom",
                                           name=f"dn_{qc}_{h}_{s}")
                        nc.vector.tensor_add(
                            denom[:, :], po[s][:, D : D + 1], esk[:, h : h + 1]
                        )
                        recip = small.tile([128, 1], F32, tag="recip",
                                           name=f"rc_{qc}_{h}_{s}")
                        nc.vector.reciprocal(recip[:, :], denom[:, :])
                        o_n = small.tile([128, D], BF16, tag="o_n",
                                         name=f"on_{qc}_{h}_{s}")
                        nc.vector.tensor_scalar_mul(
                            o_n[:, :], po[s][:, 0:D], recip[:, :]
                        )
                        o_ns[s] = o_n
                    return o_ns

                def emit_pts(qc, h, o_ns):
                    # Transpose normalized outputs into oTt; pt tiles reuse
                    # the po banks (tag-shared) to stay inside the 8-bank
                    # PSUM budget.
                    q0 = qc * QC
                    for s, o_n in o_ns.items():
                        pt = pops.tile([64, 128], BF16, tag=f"po{s % 2}",
                                       name=f"pt_{qc}_{h}_{s}")
                        nc.tensor.transpose(pt[:, :], o_n[:, :], ident[:, :])
                        jb, ro = h // 2, (h % 2) * 64
                        dst = oTt[
                            ro : ro + 64,
                            jb * S + q0 + s * 128 : jb * S + q0 + (s + 1) * 128,
                        ]
                        if s % 2 == 0:
                            nc.vector.tensor_copy(dst, pt[:, :])
                        else:
                            nc.scalar.copy(dst, pt[:, :])

                # Software pipeline: head h's score pairs are spread between
                # head h-1's AV waves (so exps drain in the shadow of AV/wo
                # work instead of stalling the score PSUM rotation), and the
                # previous chunk's wo blocks fill remaining PE slack. The PE
                # stays continuously busy, which also keeps its DVFS p-state
                # at full clock.
                WO_PER_SLOT = 6
                pending = []

                def fill_wo(n):
                    for _ in range(n):
                        if pending:
                            pending.pop(0)()

                def slot(qc, h, prev_es):
                    # Emit one pipeline slot: all score pairs for head h, then
                    # the AV waves + normalize + transposes of head h-1
                    # (prev_es), then wo filler. h == HL drains only.
                    npairs = (qc * QC // 128 + 4) // 2
                    es_pairs = []
                    if h < HL:
                        for pk in range(npairs):
                            es_pairs.append(emit_score_pair(qc, h, pk))
                    if prev_es:
                        o_a = emit_av_wave(qc, h - 1, prev_es, 0)
                        emit_pts(qc, h - 1, o_a)
                        o_b = emit_av_wave(qc, h - 1, prev_es, 1)
                        emit_pts(qc, h - 1, o_b)
                    fill_wo(WO_PER_SLOT)
                    return es_pairs

                for qc in QC_ORDER:
                    prev = None
                    for h in range(HL + 1):
                        prev = slot(qc, h, prev)
                    while pending:
                        pending.pop(0)()
                    ps_ = [0]
                    if qc != 0:
                        pending = [
                            (lambda qc=qc, mb=mb, ps_=ps_:
                             emit_wo_block(qc, mb, ps_))
                            for mb in range(MBN)
                        ]
                    else:
                        for mb in range(MBN):
                            emit_wo_block(qc, mb, ps_)

    nc.finalize()
    return nc


def make_in_maps(x, rope_cache, wq_w, wq_b, wk_w, wk_b, wv_w, wv_b, wo_w, wo_b, sinks):
    import ml_dtypes

    BF = ml_dtypes.bfloat16
    xT = np.ascontiguousarray(x[0].T).astype(BF)  # [2880, 1536]
    cosT = np.ascontiguousarray(rope_cache[:, :D].T, dtype=np.float32)
    sinT = np.ascontiguousarray(rope_cache[:, D:].T, dtype=np.float32)
    sinTs = sinT.copy()
    sinTs[: D // 2] *= -1.0
    cosT = cosT.astype(BF)
    sinTs = sinTs.astype(BF)

    in_maps = []
    for i in range(CORES):
        wq = wq_w[512 * i : 512 * (i + 1)]
        wk = wk_w[64 * i : 64 * (i + 1)]
        wv = wv_w[64 * i : 64 * (i + 1)]
        wT = np.ascontiguousarray(np.concatenate([wq, wk, wv], axis=0).T).astype(BF)
        bq = np.ascontiguousarray(
            wq_b[512 * i : 512 * (i + 1)].reshape(HL, 64).T, np.float32
        )
        bk = wk_b[64 * i : 64 * (i + 1)].reshape(64, 1).astype(np.float32)
        bv = wv_b[64 * i : 64 * (i + 1)].reshape(64, 1).astype(np.float32)
        woT = np.ascontiguousarray(wo_w[:, 512 * i : 512 * (i + 1)].T).astype(BF)
        wob8 = np.zeros((128, MBN), np.float32)
        for mb in range(MBN):
            piece = wo_b[128 * mb : 128 * (mb + 1)] / 8.0
            wob8[: len(piece), mb] = piece
        es = np.exp(sinks[HL * i : HL * (i + 1)]).astype(np.float32)
        esink = np.repeat(es[None, :], 128, axis=0).astype(np.float32)
        in_maps.append(
            {
                "xT": xT,
                "wT": wT,
                "bq": bq,
                "bk": bk,
                "bv": bv,
                "cosT": cosT,
                "sinTs": sinTs,
                "woT": woT,
                "wob8": np.ascontiguousarray(wob8),
                "esink": esink,
            }
        )
    return in_maps


_CACHE = {}


def kernel(**inputs):
    _install_hooks()
    from concourse import bass_utils

    trace = bool(int(os.environ.get("BASS_KERNEL_TRACE", "0")))
    if "nc" not in _CACHE:
        _CACHE["nc"] = build_graph()
    nc = _CACHE["nc"]

    in_maps = make_in_maps(**{k: np.asarray(v) for k, v in inputs.items()})
    res = bass_utils.run_bass_kernel_spmd(
        nc, in_maps, core_ids=list(range(CORES)), trace=trace
    )
    _EXEC_TIME_NS[0] = res.exec_time_ns

    # Assemble: piece (qc, p) covering yT rows [r0, r1) sits at rows
    # [r0/8, r1/8) of core i's 360-row shard and holds global yT rows
    # [r0 + i*(r1-r0)/8, ...) for q columns [qc*QC, qc*QC+QC).
    RS_PIECES = {2: [(0, HID)], 1: [(0, HID)], 0: [(0, 1024), (1024, HID)]}
    y = np.empty((S, HID), np.float32)
    for i in range(CORES):
        o = res.results[i]["out"].astype(np.float32)
        for qc in range(NQC):
            for r0, r1 in RS_PIECES[qc]:
                rows = (r1 - r0) // CORES
                o0 = r0 // CORES
                y[qc * QC : (qc + 1) * QC,
                  r0 + i * rows : r0 + (i + 1) * rows] = (
                    o[o0 : o0 + rows, qc * QC : (qc + 1) * QC].T
                )
    return y.reshape(1, S, HID)


def last_exec_time_ns():
    return _EXEC_TIME_NS[0]



# revision 52
# speedup vs baseline: 1.0283x; 1.0063x over previous
"""Distributed GQA attention kernel for 8 TRN2 NeuronCores.

Sharding (tensor-parallel over heads): core i owns q-heads [8i, 8i+8) and
kv-head i (GQA n_rep=8, so one kv head serves all 8 local q heads). Each core:
  1. QKV projection from the full x in bf16 (weights/activations
     pre-transposed and pre-cast host-side), fp32 PSUM accumulation. First
     weight/x pieces DMA before the constants so the PE starts immediately.
  2. RoPE on qT/kT in [d, s] layout, bf16 muls/adds (2x DVE throughput),
     sin staged sign-folded, cos/sin cached in bf16.
  3. Causal attention per head in transposed-score layout [k, q]:
     exp(scale*s) with no max subtraction (scores are O(6)), the attention
     sink enters as +exp(sink) in the denominator, and denominators ride an
     extra ones-column appended to v. Two k-blocks share one 2-bank PSUM
     tile so each exp covers [128, 1024], halving the scalar engine's fixed
     per-instruction cost. Heads are software-pipelined: head h's scores
     run while head h-1's AV waves (2 PSUM banks, tag-shared with the
     output transposes) consume already-exp'd tiles, and the previous
     chunk's wo blocks fill remaining PE slack to keep the tensor engine
     continuously busy. Triangular masks run on vector (gpsimd for the
     first chunk's even k-blocks only: its queue serializes behind
     collective triggers once ReduceScatters start).
  4. Local j-slice of the output projection -> partial yT, with wo_b/8
     folded into the eviction (sums to the exact bias across cores); wo
     blocks are interleaved between the next chunk's attention heads.
  5. Per-column-chunk bf16 ReduceScatter(add) over the 8 cores, overlapped
     with the next chunk's attention; the last chunk's scatter is row-split
     so only a small piece trails the final matmul. RS results DMA into a
     bf16 output (from the gpsimd queue, which already waits on the
     collective) and the host casts to f32, so there is no on-device bias
     tail. Host reassembles/transposes shards.
"""

import contextlib
import ctypes
import os
import sys

import numpy as np

sys.path.insert(0, "/opt/trn_rl_repo")

S = 1536
HID = 2880
D = 64
HL = 8          # local q heads per core
CORES = 8
SCQ = 512       # QKV moving chunk
NSC = S // SCQ
QC = 512        # attention q chunk
NQC = S // QC
KBN = S // 128  # 12 k blocks
VA = 68         # v_aug padded width (f32r moving dim must be 4-aligned)
CBF = 22        # full 128-row contraction blocks (2880 = 22*128 + 64)
MBN = 23        # wo output row blocks (22 full + one 64)
JBN = 4         # 512 local j rows = 4 blocks

_EXEC_TIME_NS = [None]


def _install_hooks():
    import types

    import antenv

    try:
        from antenv import axon_hooks
    except ImportError:
        axon_hooks = types.ModuleType("antenv.axon_hooks")
        _holder = {"hook": None}
        axon_hooks.set_axon_ntff_profile_hook = lambda h: _holder.update(hook=h)
        axon_hooks.get_axon_ntff_profile_hook = lambda: _holder["hook"]
        sys.modules["antenv.axon_hooks"] = axon_hooks
        antenv.axon_hooks = axon_hooks

    so_path = "/opt/axon/libaxon_pjrt.so"
    hook = None
    if os.path.exists(so_path):
        lib = ctypes.CDLL(so_path)
        if hasattr(lib, "axon_start_nrt_profile"):
            lib.axon_start_nrt_profile.argtypes = [
                ctypes.POINTER(ctypes.c_int64),
                ctypes.c_size_t,
            ]
            lib.axon_start_nrt_profile.restype = ctypes.c_int64
            lib.axon_stop_nrt_profile.argtypes = [ctypes.c_char_p]
            lib.axon_stop_nrt_profile.restype = ctypes.c_int64

            @contextlib.contextmanager
            def hook(output_dir, device_ids):
                import jax

                jax.devices()
                if device_ids:
                    ids = (ctypes.c_int64 * len(device_ids))(*device_ids)
                    rc = lib.axon_start_nrt_profile(ids, len(device_ids))
                else:
                    rc = lib.axon_start_nrt_profile(None, 0)
                if rc != 0:
                    raise RuntimeError(f"axon_start_nrt_profile rc={rc}")
                try:
                    yield
                finally:
                    n = lib.axon_stop_nrt_profile(str(output_dir).encode())
                    print(f"profile: {n} file(s) written to {output_dir}")

    axon_hooks.set_axon_ntff_profile_hook(hook)

    import concourse.bass_utils as bu

    bu.upload_artifacts = lambda tmpdir: f"local://{tmpdir}"

    if os.environ.get("BASS_LDW_OPT", "0") == "1" and not getattr(
        bu, "_ldw_patched", False
    ):
        _orig_run = bu.run_command

        def _run(cmd, *a, **k):
            cmd = [
                c.replace("--enable-ldw-opt=false", "--enable-ldw-opt=true")
                if isinstance(c, str)
                else c
                for c in cmd
            ]
            return _orig_run(cmd, *a, **k)

        bu.run_command = _run
        bu._ldw_patched = True


def build_graph():
    import concourse.mybir as mybir
    import concourse.tile as tile
    from concourse import bacc
    from concourse.masks import make_identity

    F32 = mybir.dt.float32
    BF16 = mybir.dt.bfloat16

    nc = bacc.Bacc("TRN2", target_bir_lowering=False, debug=False, num_devices=CORES)

    xT = nc.declare_dram_parameter("xT", [HID, S], BF16, isOutput=False)
    wT = nc.declare_dram_parameter("wT", [HID, 640], BF16, isOutput=False)
    bq = nc.declare_dram_parameter("bq", [64, HL], F32, isOutput=False)
    bk = nc.declare_dram_parameter("bk", [64, 1], F32, isOutput=False)
    bv = nc.declare_dram_parameter("bv", [64, 1], F32, isOutput=False)
    cosT = nc.declare_dram_parameter("cosT", [64, S], BF16, isOutput=False)
    sinTs = nc.declare_dram_parameter("sinTs", [64, S], BF16, isOutput=False)
    woT = nc.declare_dram_parameter("woT", [512, HID], BF16, isOutput=False)
    wob8 = nc.declare_dram_parameter("wob8", [128, MBN], F32, isOutput=False)
    esink = nc.declare_dram_parameter("esink", [128, HL], F32, isOutput=False)

    # qc processing order [2, 1, 0]: biggest-attention chunk first so the
    # serial ReduceScatter chain starts early; smallest chunk last.
    QC_ORDER = [2, 1, 0]
    # Row-split RS pieces per chunk: (mb_after_which_to_trigger, r0, r1).
    # Piece p covers yT rows [r0, r1); each core receives (r1-r0)/8 rows.
    # Single RS for the chunks whose transfer overlaps later compute; only the
    # last-processed chunk (qc=0) is split so less RS trails the final matmul.
    RS_PIECES = {
        2: [(10, 0, 1408), (22, 1408, HID)],
        1: [(22, 0, HID)],
        0: [(7, 0, 1024), (22, 1024, HID)],
    }
    yT_part = {}
    yT_red = {}
    for qc in range(NQC):
        yT_part[qc] = nc.dram_tensor(f"yT_part_{qc}", [HID, QC], BF16)
        for p, (_, r0, r1) in enumerate(RS_PIECES[qc]):
            yT_red[(qc, p)] = nc.dram_tensor(
                f"yT_red_{qc}_{p}", [(r1 - r0) // 8, QC], BF16
            )
    # out row ranges per piece: piece (qc, p) lands at rows r0//8..r1//8 of
    # this core's 360-row shard; bf16 (host casts to f32).
    out = nc.declare_dram_parameter("out", [360, S], BF16, isOutput=True)

    Exp = mybir.ActivationFunctionType.Exp

    # QKV contraction pieces: groups of 128-row c-blocks (22 full + one 64-row)
    PIECES = [(0, 4), (4, 4), (8, 4), (12, 4), (16, 4), (20, 2)]

    with tile.TileContext(nc) as tc:
        with contextlib.ExitStack() as stack:
            consts = stack.enter_context(tc.tile_pool(name="consts", bufs=1))
            qkvout = stack.enter_context(tc.tile_pool(name="qkvout", bufs=1))
            small = stack.enter_context(tc.tile_pool(name="small", bufs=6))
            ytp = stack.enter_context(tc.tile_pool(name="ytp", bufs=4))

            bqt = consts.tile([64, HL], F32, tag="bq")
            bkt = consts.tile([64, 1], F32, tag="bk")
            bvt = consts.tile([64, 1], F32, tag="bv")
            cost = consts.tile([64, S], BF16, tag="cos")
            sint = consts.tile([64, S], BF16, tag="sin")
            wob8t = consts.tile([128, MBN], F32, tag="wob8")
            esk = consts.tile([128, HL], F32, tag="esk")
            ident_f = consts.tile([128, 128], F32, tag="ident_f")
            ident = consts.tile([128, 128], BF16, tag="ident")
            ones = consts.tile([128, 1], F32, tag="ones")
            # const DMAs are issued inside the QKV block, after the first
            # weight/x pieces, so the PE's first matmul data arrives first
            const_dmas = [(bqt, bq), (bkt, bk), (bvt, bv), (cost, cosT),
                          (sint, sinTs), (wob8t, wob8), (esk, esink)]
            make_identity(nc, ident_f[:, :])
            nc.vector.tensor_copy(ident[:, :], ident_f[:, :])
            nc.vector.memset(ones[:, :], 1.0)
            tri = consts.tile([128, 128], BF16, tag="tri")
            nc.vector.memset(tri[:, :], 1.0)
            nc.gpsimd.affine_select(
                out=tri[:, :], in_=tri[:, :],
                compare_op=mybir.AluOpType.is_ge,
                fill=0.0, base=0, pattern=[[1, 128]], channel_multiplier=-1,
            )

            qq = qkvout.tile([64, HL * S], BF16, tag="qq")
            kh = qkvout.tile([64, S], BF16, tag="kh")
            vT = qkvout.tile([64, S], BF16, tag="vT")
            vaug = qkvout.tile([128, KBN * VA], BF16, tag="vaug")

            # ---------------- QKV projection (piece-streamed) ----------------
            # Loop pieces outer / nb inner with 5 live PSUM banks so the first
            # matmul only needs the first weight+x piece DMAs, starting the PE
            # ~25us earlier. Weight and first-chunk x DMAs are interleaved so
            # earliest-needed data arrives first.
            with (
                tc.tile_pool(name="wtp", bufs=1) as wtp,
                tc.tile_pool(name="xcp", bufs=3) as xcp,
                tc.tile_pool(name="qkps", bufs=4, space="PSUM") as qkps,
                tc.tile_pool(name="rtmp", bufs=3) as rtmp,
            ):
                wts = []
                xps0 = []

                def dma_wt(pc):
                    cb0, ncb = PIECES[pc]
                    wt_pc = wtp.tile([128, ncb * 640], BF16, tag=f"wt{pc}",
                                     name=f"wt{pc}")
                    nc.sync.dma_start(
                        out=wt_pc[:, :].rearrange("p (cb n) -> p cb n", cb=ncb),
                        in_=wT[cb0 * 128 : (cb0 + ncb) * 128, :].rearrange(
                            "(cb p) n -> p cb n", p=128
                        ),
                    )
                    wts.append(wt_pc)

                def dma_xp(pc, sc):
                    cb0, ncb = PIECES[pc]
                    c0 = sc * SCQ
                    xp = xcp.tile([128, ncb * SCQ], BF16, tag=f"xp{pc}",
                                  name=f"xp{pc}_{sc}")
                    nc.sync.dma_start(
                        out=xp[:, :].rearrange("p (cb s) -> p cb s", cb=ncb),
                        in_=xT[cb0 * 128 : (cb0 + ncb) * 128, c0 : c0 + SCQ].rearrange(
                            "(cb p) s -> p cb s", p=128
                        ),
                    )
                    return xp

                for pc in range(len(PIECES)):
                    dma_wt(pc)
                    xps0.append(dma_xp(pc, 0))
                    if pc == 0:
                        for t, src_ in const_dmas[:3]:
                            nc.sync.dma_start(out=t[:, :], in_=src_[:, :])
                    elif pc == 1:
                        for t, src_ in const_dmas[3:]:
                            nc.sync.dma_start(out=t[:, :], in_=src_[:, :])
                wt2 = wtp.tile([64, 640], BF16, tag="wtail")
                nc.sync.dma_start(out=wt2[:, :], in_=wT[CBF * 128 : HID, :])

                for sc in range(NSC):
                    c0 = sc * SCQ
                    if sc == 0:
                        xps = xps0
                    else:
                        xps = [dma_xp(pc, sc) for pc in range(len(PIECES))]
                    xc2 = xcp.tile([64, SCQ], BF16, tag="xc2", name=f"xc2_{sc}")
                    nc.sync.dma_start(
                        out=xc2[:, :], in_=xT[CBF * 128 : HID, c0 : c0 + SCQ]
                    )

                    for nb in range(5):
                        p = qkps.tile([128, SCQ], F32, tag="qkv", name=f"qk{nb}_{sc}")
                        for pc, (cb0, ncb) in enumerate(PIECES):
                            for i in range(ncb):
                                nc.tensor.matmul(
                                    p[:, :],
                                    wts[pc][:, i * 640 + nb * 128 : i * 640 + (nb + 1) * 128],
                                    xps[pc][:, i * SCQ : (i + 1) * SCQ],
                                    start=(pc == 0 and i == 0),
                                    stop=False,
                                )
                        nc.tensor.matmul(
                            p[:, :],
                            wt2[:, nb * 128 : (nb + 1) * 128],
                            xc2[:, :],
                            start=False,
                            stop=True,
                        )
                        if nb < 4:
                            for half in range(2):
                                h = 2 * nb + half
                                hb = 64 * half
                                qb = rtmp.tile([64, SCQ], BF16, tag="qb",
                                               name=f"qb_{sc}_{nb}_{half}")
                                nc.vector.tensor_scalar_add(
                                    qb[:, :], p[hb : hb + 64, :], bqt[:, h : h + 1]
                                )
                                rot = rtmp.tile([64, SCQ], BF16, tag="rot",
                                                name=f"rot_{sc}_{nb}_{half}")
                                nc.scalar.copy(rot[0:32, :], qb[32:64, :])
                                nc.scalar.copy(rot[32:64, :], qb[0:32, :])
                                nc.vector.tensor_mul(
                                    qb[:, :], qb[:, :], cost[:, c0 : c0 + SCQ]
                                )
                                nc.vector.tensor_mul(
                                    rot[:, :], rot[:, :], sint[:, c0 : c0 + SCQ]
                                )
                                nc.vector.tensor_add(
                                    qq[:, h * S + c0 : h * S + c0 + SCQ],
                                    qb[:, :],
                                    rot[:, :],
                                )
                        else:
                            kb_ = rtmp.tile([64, SCQ], BF16, tag="qb",
                                            name=f"kb_{sc}")
                            nc.vector.tensor_scalar_add(
                                kb_[:, :], p[0:64, :], bkt[:, 0:1]
                            )
                            rot = rtmp.tile([64, SCQ], BF16, tag="rot",
                                            name=f"krot_{sc}")
                            nc.scalar.copy(rot[0:32, :], kb_[32:64, :])
                            nc.scalar.copy(rot[32:64, :], kb_[0:32, :])
                            nc.vector.tensor_mul(
                                kb_[:, :], kb_[:, :], cost[:, c0 : c0 + SCQ]
                            )
                            nc.vector.tensor_mul(
                                rot[:, :], rot[:, :], sint[:, c0 : c0 + SCQ]
                            )
                            nc.vector.tensor_add(
                                kh[:, c0 : c0 + SCQ], kb_[:, :], rot[:, :]
                            )
                            nc.vector.tensor_scalar_add(
                                vT[:, c0 : c0 + SCQ], p[64:128, :], bvt[:, 0:1]
                            )

            # ---------------- v transpose + ones column ----------------
            with tc.tile_pool(name="vtp", bufs=2, space="PSUM") as vtp:
                for kb in range(KBN):
                    pv = vtp.tile([128, D], BF16, tag="pv", name=f"pv{kb}")
                    nc.tensor.transpose(
                        pv[:, :], vT[:, kb * 128 : (kb + 1) * 128], ident[0:64, 0:64]
                    )
                    nc.vector.tensor_copy(vaug[:, kb * VA : kb * VA + 64], pv[:, :])
                    for oc in range(64, VA):
                        nc.vector.tensor_copy(
                            vaug[:, kb * VA + oc : kb * VA + oc + 1], ones[:, :]
                        )

            # ---------------- attention + wo + chunked ReduceScatter ----------------
            with (
                tc.tile_pool(name="oTp", bufs=1) as oTp,
                tc.tile_pool(name="woTp", bufs=1) as woTp,
                tc.tile_pool(name="esp", bufs=14) as esp,
                tc.tile_pool(name="scps", bufs=2, space="PSUM") as scps,
                tc.tile_pool(name="pops", bufs=1, space="PSUM") as pops,
                tc.tile_pool(name="wops", bufs=2, space="PSUM") as wops,
            ):
                oTt = oTp.tile([128, JBN * S], BF16, tag="oT")
                woTt = woTp.tile([128, JBN * HID], BF16, tag="woT")
                nc.sync.dma_start(
                    out=woTt[:, :].rearrange("p (jb m) -> p jb m", jb=JBN),
                    in_=woT[:, :].rearrange("(jb p) m -> p jb m", p=128),
                )

                deferred_out = []

                def emit_wo_block(qc, mb, piece_state, defer_out=False):
                    # One wo output-row block: 4 accumulating matmuls over jb,
                    # bias-add eviction (vector), DMA to yT_part, and the RS
                    # trigger + out-DMA when this mb completes a row piece.
                    q0 = qc * QC
                    rows = 128 if mb < CBF else 64
                    pw = wops.tile([128, QC], F32, tag="wo",
                                   name=f"pw_{qc}_{mb}")
                    for jb in range(JBN):
                        nc.tensor.matmul(
                            pw[0:rows, :],
                            woTt[:, jb * HID + mb * 128 : jb * HID + mb * 128 + rows],
                            oTt[:, jb * S + q0 : jb * S + q0 + QC],
                            start=(jb == 0),
                            stop=(jb == JBN - 1),
                        )
                    yt = ytp.tile([128, QC], BF16, tag="ytb",
                                  name=f"yt_{qc}_{mb}")
                    if mb % 2 == 0:
                        nc.vector.tensor_scalar_add(
                            yt[0:rows, :], pw[0:rows, :],
                            wob8t[0:rows, mb : mb + 1],
                        )
                    else:
                        nc.scalar.activation(
                            yt[0:rows, :], pw[0:rows, :],
                            mybir.ActivationFunctionType.Identity,
                            bias=wob8t[0:rows, mb : mb + 1],
                        )
                    nc.sync.dma_start(
                        out=yT_part[qc][mb * 128 : mb * 128 + rows, :],
                        in_=yt[0:rows, :],
                    )
                    piece = piece_state[0]
                    if (piece < len(RS_PIECES[qc])
                            and mb == RS_PIECES[qc][piece][0]):
                        _, r0, r1 = RS_PIECES[qc][piece]
                        nc.gpsimd.collective_compute(
                            "ReduceScatter",
                            mybir.AluOpType.add,
                            replica_groups=[list(range(CORES))],
                            ins=[yT_part[qc][r0:r1, :].opt()],
                            outs=[yT_red[(qc, piece)].ap().opt()],
                        )
                        o0 = r0 // CORES
                        nrows = (r1 - r0) // CORES
                        # issue from gpsimd: this DMA waits on the collective,
                        # and must not block the sync queue's evictions. For
                        # the last chunk, defer it past the remaining RS
                        # triggers so they launch back-to-back.
                        args = (out[o0 : o0 + nrows, q0 : q0 + QC],
                                yT_red[(qc, piece)].ap())
                        if defer_out:
                            deferred_out.append(args)
                        else:
                            nc.gpsimd.dma_start(out=args[0], in_=args[1])
                        piece_state[0] += 1

                def emit_score_pair(qc, h, pk):
                    # Two k-blocks into one 2-bank PSUM tile, one exp over
                    # [128, 1024] (halves the scalar engine's fixed cost).
                    # Diagonal k-blocks compute full width; the sub-diagonal
                    # garbage is never read (AV skips s < j) and only the
                    # diagonal 128-block needs the triangular mask.
                    q0 = qc * QC
                    qb0 = q0 // 128
                    ps_s = scps.tile([128, 2 * QC], F32, tag="scores",
                                     name=f"ps_{qc}_{h}_{pk}")
                    for half in range(2):
                        kb = 2 * pk + half
                        j = kb - qb0
                        w0 = 128 * j if j > 0 else 0
                        nc.tensor.matmul(
                            ps_s[:, half * QC + w0 : (half + 1) * QC],
                            kh[:, kb * 128 : (kb + 1) * 128],
                            qq[:, h * S + q0 + w0 : h * S + q0 + QC],
                            start=True,
                            stop=True,
                        )
                    es = esp.tile([128, 2 * QC], BF16, tag="es",
                                  name=f"es_{qc}_{h}_{pk}")
                    nc.scalar.activation(es[:, :], ps_s[:, :], Exp, scale=0.125)
                    for half in range(2):
                        kb = 2 * pk + half
                        j = kb - qb0
                        if j >= 0:
                            dc = half * QC + 128 * j
                            # gpsimd is collective-free until the first RS
                            # trigger (mid-attn of the second chunk), so the
                            # first chunk's even masks can run there
                            if qc == QC_ORDER[0] and kb % 2 == 0:
                                nc.gpsimd.affine_select(
                                    out=es[:, dc : dc + 128],
                                    in_=es[:, dc : dc + 128],
                                    compare_op=mybir.AluOpType.is_ge,
                                    fill=0.0,
                                    base=0,
                                    pattern=[[1, 128]],
                                    channel_multiplier=-1,
                                )
                            else:
                                nc.vector.tensor_mul(
                                    es[:, dc : dc + 128],
                                    es[:, dc : dc + 128],
                                    tri[:, :],
                                )
                    return es

                def emit_av_wave(qc, h, es_pairs, wave):
                    # AV accumulation for q sub-blocks (2*wave, 2*wave+1) plus
                    # the vector normalize chain; the pt transposes are
                    # deferred (emit_pts) so the PE isn't parked behind the
                    # vector chain.
                    qb0 = qc * QC // 128
                    po = {}
                    for s in (2 * wave, 2 * wave + 1):
                        po[s] = pops.tile([128, VA], F32, tag=f"po{s % 2}",
                                          name=f"po{s}_{qc}_{h}")
                        for kb in range(qb0 + s + 1):
                            nc.tensor.matmul(
                                po[s][:, :],
                                es_pairs[kb // 2][:, (kb % 2) * QC + s * 128 :
                                                  (kb % 2) * QC + s * 128 + 128],
                                vaug[:, kb * VA : (kb + 1) * VA],
                                start=(kb == 0),
                                stop=(kb == qb0 + s),
                            )
                    o_ns = {}
                    for s in (2 * wave, 2 * wave + 1):
                        denom = small.tile([128, 1], F32, tag="denom",
                                           name=f"dn_{qc}_{h}_{s}")
                        nc.vector.tensor_add(
                            denom[:, :], po[s][:, D : D + 1], esk[:, h : h + 1]
                        )
                        recip = small.tile([128, 1], F32, tag="recip",
                                           name=f"rc_{qc}_{h}_{s}")
                        nc.vector.reciprocal(recip[:, :], denom[:, :])
                        o_n = small.tile([128, D], BF16, tag="o_n",
                                         name=f"on_{qc}_{h}_{s}")
                        nc.vector.tensor_scalar_mul(
                            o_n[:, :], po[s][:, 0:D], recip[:, :]
                        )
                        o_ns[s] = o_n
                    return o_ns

                def emit_pts(qc, h, o_ns):
                    # Transpose normalized outputs into oTt; pt tiles reuse
                    # the po banks (tag-shared) to stay inside the 8-bank
                    # PSUM budget.
                    q0 = qc * QC
                    for s, o_n in o_ns.items():
                        pt = pops.tile([64, 128], BF16, tag=f"po{s % 2}",
                                       name=f"pt_{qc}_{h}_{s}")
                        nc.tensor.transpose(pt[:, :], o_n[:, :], ident[:, :])
                        jb, ro = h // 2, (h % 2) * 64
                        dst = oTt[
                            ro : ro + 64,
                            jb * S + q0 + s * 128 : jb * S + q0 + (s + 1) * 128,
                        ]
                        if s % 2 == 0:
                            nc.vector.tensor_copy(dst, pt[:, :])
                        else:
                            nc.scalar.copy(dst, pt[:, :])

                # Software pipeline: head h's score pairs are spread between
                # head h-1's AV waves (so exps drain in the shadow of AV/wo
                # work instead of stalling the score PSUM rotation), and the
                # previous chunk's wo blocks fill remaining PE slack. The PE
                # stays continuously busy, which also keeps its DVFS p-state
                # at full clock.
                WO_PER_SLOT = 6
                pending = []

                def fill_wo(n):
                    for _ in range(n):
                        if pending:
                            pending.pop(0)()

                def slot(qc, h, prev_es):
                    # Emit one pipeline slot: all score pairs for head h, then
                    # the AV waves + normalize + transposes of head h-1
                    # (prev_es), then wo filler. h == HL drains only.
                    npairs = (qc * QC // 128 + 4) // 2
                    es_pairs = []
                    if h < HL:
                        for pk in range(npairs):
                            es_pairs.append(emit_score_pair(qc, h, pk))
                    if prev_es:
                        o_a = emit_av_wave(qc, h - 1, prev_es, 0)
                        emit_pts(qc, h - 1, o_a)
                        o_b = emit_av_wave(qc, h - 1, prev_es, 1)
                        emit_pts(qc, h - 1, o_b)
                    fill_wo(WO_PER_SLOT)
                    return es_pairs

                for qc in QC_ORDER:
                    prev = None
                    for h in range(HL + 1):
                        prev = slot(qc, h, prev)
                    while pending:
                        pending.pop(0)()
                    ps_ = [0]
                    if qc != 0:
                        pending = [
                            (lambda qc=qc, mb=mb, ps_=ps_:
                             emit_wo_block(qc, mb, ps_))
                            for mb in range(MBN)
                        ]
                    else:
                        for mb in range(MBN):
                            emit_wo_block(qc, mb, ps_, defer_out=True)
                        for o_ap, in_ap in deferred_out:
                            nc.gpsimd.dma_start(out=o_ap, in_=in_ap)

    nc.finalize()
    return nc


def make_in_maps(x, rope_cache, wq_w, wq_b, wk_w, wk_b, wv_w, wv_b, wo_w, wo_b, sinks):
    import ml_dtypes

    BF = ml_dtypes.bfloat16
    xT = np.ascontiguousarray(x[0].T).astype(BF)  # [2880, 1536]
    cosT = np.ascontiguousarray(rope_cache[:, :D].T, dtype=np.float32)
    sinT = np.ascontiguousarray(rope_cache[:, D:].T, dtype=np.float32)
    sinTs = sinT.copy()
    sinTs[: D // 2] *= -1.0
    cosT = cosT.astype(BF)
    sinTs = sinTs.astype(BF)

    in_maps = []
    for i in range(CORES):
        wq = wq_w[512 * i : 512 * (i + 1)]
        wk = wk_w[64 * i : 64 * (i + 1)]
        wv = wv_w[64 * i : 64 * (i + 1)]
        wT = np.ascontiguousarray(np.concatenate([wq, wk, wv], axis=0).T).astype(BF)
        bq = np.ascontiguousarray(
            wq_b[512 * i : 512 * (i + 1)].reshape(HL, 64).T, np.float32
        )
        bk = wk_b[64 * i : 64 * (i + 1)].reshape(64, 1).astype(np.float32)
        bv = wv_b[64 * i : 64 * (i + 1)].reshape(64, 1).astype(np.float32)
        woT = np.ascontiguousarray(wo_w[:, 512 * i : 512 * (i + 1)].T).astype(BF)
        wob8 = np.zeros((128, MBN), np.float32)
        for mb in range(MBN):
            piece = wo_b[128 * mb : 128 * (mb + 1)] / 8.0
            wob8[: len(piece), mb] = piece
        es = np.exp(sinks[HL * i : HL * (i + 1)]).astype(np.float32)
        esink = np.repeat(es[None, :], 128, axis=0).astype(np.float32)
        in_maps.append(
            {
                "xT": xT,
                "wT": wT,
                "bq": bq,
                "bk": bk,
                "bv": bv,
                "cosT": cosT,
                "sinTs": sinTs,
                "woT": woT,
                "wob8": np.ascontiguousarray(wob8),
                "esink": esink,
            }
        )
    return in_maps


_CACHE = {}


def kernel(**inputs):
    _install_hooks()
    from concourse import bass_utils

    trace = bool(int(os.environ.get("BASS_KERNEL_TRACE", "0")))
    if "nc" not in _CACHE:
        _CACHE["nc"] = build_graph()
    nc = _CACHE["nc"]

    in_maps = make_in_maps(**{k: np.asarray(v) for k, v in inputs.items()})
    res = bass_utils.run_bass_kernel_spmd(
        nc, in_maps, core_ids=list(range(CORES)), trace=trace
    )
    _EXEC_TIME_NS[0] = res.exec_time_ns

    # Assemble: piece (qc, p) covering yT rows [r0, r1) sits at rows
    # [r0/8, r1/8) of core i's 360-row shard and holds global yT rows
    # [r0 + i*(r1-r0)/8, ...) for q columns [qc*QC, qc*QC+QC).
    RS_PIECES = {2: [(0, 1408), (1408, HID)], 1: [(0, HID)],
                 0: [(0, 1024), (1024, HID)]}
    y = np.empty((S, HID), np.float32)
    for i in range(CORES):
        o = res.results[i]["out"].astype(np.float32)
        for qc in range(NQC):
            for r0, r1 in RS_PIECES[qc]:
                rows = (r1 - r0) // CORES
                o0 = r0 // CORES
                y[qc * QC : (qc + 1) * QC,
                  r0 + i * rows : r0 + (i + 1) * rows] = (
                    o[o0 : o0 + rows, qc * QC : (qc + 1) * QC].T
                )
    return y.reshape(1, S, HID)


def last_exec_time_ns():
    return _EXEC_TIME_NS[0]



# revision 54
# speedup vs baseline: 1.0347x; 1.0062x over previous
"""Distributed GQA attention kernel for 8 TRN2 NeuronCores.

Sharding (tensor-parallel over heads): core i owns q-heads [8i, 8i+8) and
kv-head i (GQA n_rep=8, so one kv head serves all 8 local q heads). Each core:
  1. QKV projection from the full x in bf16 (weights/activations
     pre-transposed and pre-cast host-side), fp32 PSUM accumulation. First
     weight/x pieces DMA before the constants so the PE starts immediately.
  2. RoPE on qT/kT in [d, s] layout, bf16 muls/adds (2x DVE throughput),
     sin staged sign-folded, cos/sin cached in bf16.
  3. Causal attention per head in transposed-score layout [k, q]:
     exp(scale*s) with no max subtraction (scores are O(6)), the attention
     sink enters as +exp(sink) in the denominator, and denominators ride an
     extra ones-column appended to v. Two k-blocks share one 2-bank PSUM
     tile so each exp covers [128, 1024], halving the scalar engine's fixed
     per-instruction cost. Heads are software-pipelined: head h's scores
     run while head h-1's AV waves (2 PSUM banks, tag-shared with the
     output transposes) consume already-exp'd tiles, and the previous
     chunk's wo blocks fill remaining PE slack to keep the tensor engine
     continuously busy. Triangular masks run on vector (gpsimd for the
     first chunk's even k-blocks only: its queue serializes behind
     collective triggers once ReduceScatters start).
  4. Local j-slice of the output projection -> partial yT, with wo_b/8
     folded into the eviction (sums to the exact bias across cores); wo
     blocks are interleaved between the next chunk's attention heads.
  5. Per-column-chunk bf16 ReduceScatter(add) over the 8 cores, overlapped
     with the next chunk's attention; the last chunk's scatter is row-split
     so only a small piece trails the final matmul. RS results DMA into a
     bf16 output (from the gpsimd queue, which already waits on the
     collective) and the host casts to f32, so there is no on-device bias
     tail. Host reassembles/transposes shards.
"""

import contextlib
import ctypes
import os
import sys

import numpy as np

sys.path.insert(0, "/opt/trn_rl_repo")

S = 1536
HID = 2880
D = 64
HL = 8          # local q heads per core
CORES = 8
SCQ = 512       # QKV moving chunk
NSC = S // SCQ
QC = 512        # attention q chunk
NQC = S // QC
KBN = S // 128  # 12 k blocks
VA = 68         # v_aug padded width (f32r moving dim must be 4-aligned)
CBF = 22        # full 128-row contraction blocks (2880 = 22*128 + 64)
MBN = 23        # wo output row blocks (22 full + one 64)
JBN = 4         # 512 local j rows = 4 blocks

_EXEC_TIME_NS = [None]


def _install_hooks():
    import types

    import antenv

    try:
        from antenv import axon_hooks
    except ImportError:
        axon_hooks = types.ModuleType("antenv.axon_hooks")
        _holder = {"hook": None}
        axon_hooks.set_axon_ntff_profile_hook = lambda h: _holder.update(hook=h)
        axon_hooks.get_axon_ntff_profile_hook = lambda: _holder["hook"]
        sys.modules["antenv.axon_hooks"] = axon_hooks
        antenv.axon_hooks = axon_hooks

    so_path = "/opt/axon/libaxon_pjrt.so"
    hook = None
    if os.path.exists(so_path):
        lib = ctypes.CDLL(so_path)
        if hasattr(lib, "axon_start_nrt_profile"):
            lib.axon_start_nrt_profile.argtypes = [
                ctypes.POINTER(ctypes.c_int64),
                ctypes.c_size_t,
            ]
            lib.axon_start_nrt_profile.restype = ctypes.c_int64
            lib.axon_stop_nrt_profile.argtypes = [ctypes.c_char_p]
            lib.axon_stop_nrt_profile.restype = ctypes.c_int64

            @contextlib.contextmanager
            def hook(output_dir, device_ids):
                import jax

                jax.devices()
                if device_ids:
                    ids = (ctypes.c_int64 * len(device_ids))(*device_ids)
                    rc = lib.axon_start_nrt_profile(ids, len(device_ids))
                else:
                    rc = lib.axon_start_nrt_profile(None, 0)
                if rc != 0:
                    raise RuntimeError(f"axon_start_nrt_profile rc={rc}")
                try:
                    yield
                finally:
                    n = lib.axon_stop_nrt_profile(str(output_dir).encode())
                    print(f"profile: {n} file(s) written to {output_dir}")

    axon_hooks.set_axon_ntff_profile_hook(hook)

    import concourse.bass_utils as bu

    bu.upload_artifacts = lambda tmpdir: f"local://{tmpdir}"

    if os.environ.get("BASS_LDW_OPT", "0") == "1" and not getattr(
        bu, "_ldw_patched", False
    ):
        _orig_run = bu.run_command

        def _run(cmd, *a, **k):
            cmd = [
                c.replace("--enable-ldw-opt=false", "--enable-ldw-opt=true")
                if isinstance(c, str)
                else c
                for c in cmd
            ]
            return _orig_run(cmd, *a, **k)

        bu.run_command = _run
        bu._ldw_patched = True


def build_graph():
    import concourse.mybir as mybir
    import concourse.tile as tile
    from concourse import bacc
    from concourse.masks import make_identity

    F32 = mybir.dt.float32
    BF16 = mybir.dt.bfloat16

    nc = bacc.Bacc("TRN2", target_bir_lowering=False, debug=False, num_devices=CORES)

    xT = nc.declare_dram_parameter("xT", [HID, S], BF16, isOutput=False)
    wT = nc.declare_dram_parameter("wT", [HID, 640], BF16, isOutput=False)
    bq = nc.declare_dram_parameter("bq", [64, HL], F32, isOutput=False)
    bk = nc.declare_dram_parameter("bk", [64, 1], F32, isOutput=False)
    bv = nc.declare_dram_parameter("bv", [64, 1], F32, isOutput=False)
    cosT = nc.declare_dram_parameter("cosT", [64, S], BF16, isOutput=False)
    sinTs = nc.declare_dram_parameter("sinTs", [64, S], BF16, isOutput=False)
    woT = nc.declare_dram_parameter("woT", [512, HID], BF16, isOutput=False)
    wob8 = nc.declare_dram_parameter("wob8", [128, MBN], F32, isOutput=False)
    esink = nc.declare_dram_parameter("esink", [128, HL], F32, isOutput=False)

    # qc processing order [2, 1, 0]: biggest-attention chunk first so the
    # serial ReduceScatter chain starts early; smallest chunk last.
    QC_ORDER = [2, 1, 0]
    # Row-split RS pieces per chunk: (mb_after_which_to_trigger, r0, r1).
    # Piece p covers yT rows [r0, r1); each core receives (r1-r0)/8 rows.
    # Single RS for the chunks whose transfer overlaps later compute; only the
    # last-processed chunk (qc=0) is split so less RS trails the final matmul.
    RS_PIECES = {
        2: [(10, 0, 1408), (22, 1408, HID)],
        1: [(22, 0, HID)],
        0: [(7, 0, 1024), (22, 1024, HID)],
    }
    yT_part = {}
    yT_red = {}
    for qc in range(NQC):
        yT_part[qc] = nc.dram_tensor(f"yT_part_{qc}", [HID, QC], BF16)
        for p, (_, r0, r1) in enumerate(RS_PIECES[qc]):
            yT_red[(qc, p)] = nc.dram_tensor(
                f"yT_red_{qc}_{p}", [(r1 - r0) // 8, QC], BF16
            )
    # out row ranges per piece: piece (qc, p) lands at rows r0//8..r1//8 of
    # this core's 360-row shard; bf16 (host casts to f32).
    out = nc.declare_dram_parameter("out", [360, S], BF16, isOutput=True)

    Exp = mybir.ActivationFunctionType.Exp

    # QKV contraction pieces: groups of 128-row c-blocks (22 full + one 64-row)
    PIECES = [(0, 4), (4, 4), (8, 4), (12, 4), (16, 4), (20, 2)]

    with tile.TileContext(nc) as tc:
        with contextlib.ExitStack() as stack:
            consts = stack.enter_context(tc.tile_pool(name="consts", bufs=1))
            qkvout = stack.enter_context(tc.tile_pool(name="qkvout", bufs=1))
            small = stack.enter_context(tc.tile_pool(name="small", bufs=6))
            ytp = stack.enter_context(tc.tile_pool(name="ytp", bufs=4))

            bqt = consts.tile([64, HL], F32, tag="bq")
            bkt = consts.tile([64, 1], F32, tag="bk")
            bvt = consts.tile([64, 1], F32, tag="bv")
            cost = consts.tile([64, S], BF16, tag="cos")
            sint = consts.tile([64, S], BF16, tag="sin")
            wob8t = consts.tile([128, MBN], F32, tag="wob8")
            esk = consts.tile([128, HL], F32, tag="esk")
            ident_f = consts.tile([128, 128], F32, tag="ident_f")
            ident = consts.tile([128, 128], BF16, tag="ident")
            ones = consts.tile([128, 1], F32, tag="ones")
            # const DMAs are issued inside the QKV block, after the first
            # weight/x pieces, so the PE's first matmul data arrives first
            const_dmas = [(bqt, bq), (bkt, bk), (bvt, bv), (cost, cosT),
                          (sint, sinTs), (wob8t, wob8), (esk, esink)]
            make_identity(nc, ident_f[:, :])
            nc.vector.tensor_copy(ident[:, :], ident_f[:, :])
            nc.vector.memset(ones[:, :], 1.0)
            tri = consts.tile([128, 128], BF16, tag="tri")
            nc.vector.memset(tri[:, :], 1.0)
            nc.gpsimd.affine_select(
                out=tri[:, :], in_=tri[:, :],
                compare_op=mybir.AluOpType.is_ge,
                fill=0.0, base=0, pattern=[[1, 128]], channel_multiplier=-1,
            )

            qq = qkvout.tile([64, HL * S], BF16, tag="qq")
            kh = qkvout.tile([64, S], BF16, tag="kh")
            vT = qkvout.tile([64, S], BF16, tag="vT")
            vaug = qkvout.tile([128, KBN * VA], BF16, tag="vaug")

            # ---------------- QKV projection (piece-streamed) ----------------
            # Loop pieces outer / nb inner with 5 live PSUM banks so the first
            # matmul only needs the first weight+x piece DMAs, starting the PE
            # ~25us earlier. Weight and first-chunk x DMAs are interleaved so
            # earliest-needed data arrives first.
            with (
                tc.tile_pool(name="wtp", bufs=1) as wtp,
                tc.tile_pool(name="xcp", bufs=3) as xcp,
                tc.tile_pool(name="qkps", bufs=4, space="PSUM") as qkps,
                tc.tile_pool(name="rtmp", bufs=3) as rtmp,
            ):
                wts = []
                xps0 = []

                def dma_wt(pc):
                    cb0, ncb = PIECES[pc]
                    wt_pc = wtp.tile([128, ncb * 640], BF16, tag=f"wt{pc}",
                                     name=f"wt{pc}")
                    nc.sync.dma_start(
                        out=wt_pc[:, :].rearrange("p (cb n) -> p cb n", cb=ncb),
                        in_=wT[cb0 * 128 : (cb0 + ncb) * 128, :].rearrange(
                            "(cb p) n -> p cb n", p=128
                        ),
                    )
                    wts.append(wt_pc)

                def dma_xp(pc, sc):
                    cb0, ncb = PIECES[pc]
                    c0 = sc * SCQ
                    xp = xcp.tile([128, ncb * SCQ], BF16, tag=f"xp{pc}",
                                  name=f"xp{pc}_{sc}")
                    nc.sync.dma_start(
                        out=xp[:, :].rearrange("p (cb s) -> p cb s", cb=ncb),
                        in_=xT[cb0 * 128 : (cb0 + ncb) * 128, c0 : c0 + SCQ].rearrange(
                            "(cb p) s -> p cb s", p=128
                        ),
                    )
                    return xp

                for pc in range(len(PIECES)):
                    dma_wt(pc)
                    xps0.append(dma_xp(pc, 0))
                    if pc == 0:
                        for t, src_ in const_dmas[:3]:
                            nc.sync.dma_start(out=t[:, :], in_=src_[:, :])
                    elif pc == 1:
                        for t, src_ in const_dmas[3:]:
                            nc.sync.dma_start(out=t[:, :], in_=src_[:, :])
                wt2 = wtp.tile([64, 640], BF16, tag="wtail")
                nc.sync.dma_start(out=wt2[:, :], in_=wT[CBF * 128 : HID, :])

                for sc in range(NSC):
                    c0 = sc * SCQ
                    if sc == 0:
                        xps = xps0
                    else:
                        xps = [dma_xp(pc, sc) for pc in range(len(PIECES))]
                    xc2 = xcp.tile([64, SCQ], BF16, tag="xc2", name=f"xc2_{sc}")
                    nc.sync.dma_start(
                        out=xc2[:, :], in_=xT[CBF * 128 : HID, c0 : c0 + SCQ]
                    )

                    for nb in range(5):
                        p = qkps.tile([128, SCQ], F32, tag="qkv", name=f"qk{nb}_{sc}")
                        for pc, (cb0, ncb) in enumerate(PIECES):
                            for i in range(ncb):
                                nc.tensor.matmul(
                                    p[:, :],
                                    wts[pc][:, i * 640 + nb * 128 : i * 640 + (nb + 1) * 128],
                                    xps[pc][:, i * SCQ : (i + 1) * SCQ],
                                    start=(pc == 0 and i == 0),
                                    stop=False,
                                )
                        nc.tensor.matmul(
                            p[:, :],
                            wt2[:, nb * 128 : (nb + 1) * 128],
                            xc2[:, :],
                            start=False,
                            stop=True,
                        )
                        if nb < 4:
                            for half in range(2):
                                h = 2 * nb + half
                                hb = 64 * half
                                qb = rtmp.tile([64, SCQ], BF16, tag="qb",
                                               name=f"qb_{sc}_{nb}_{half}")
                                nc.vector.tensor_scalar_add(
                                    qb[:, :], p[hb : hb + 64, :], bqt[:, h : h + 1]
                                )
                                rot = rtmp.tile([64, SCQ], BF16, tag="rot",
                                                name=f"rot_{sc}_{nb}_{half}")
                                nc.scalar.copy(rot[0:32, :], qb[32:64, :])
                                nc.scalar.copy(rot[32:64, :], qb[0:32, :])
                                nc.vector.tensor_mul(
                                    qb[:, :], qb[:, :], cost[:, c0 : c0 + SCQ]
                                )
                                nc.vector.tensor_mul(
                                    rot[:, :], rot[:, :], sint[:, c0 : c0 + SCQ]
                                )
                                nc.vector.tensor_add(
                                    qq[:, h * S + c0 : h * S + c0 + SCQ],
                                    qb[:, :],
                                    rot[:, :],
                                )
                        else:
                            kb_ = rtmp.tile([64, SCQ], BF16, tag="qb",
                                            name=f"kb_{sc}")
                            nc.vector.tensor_scalar_add(
                                kb_[:, :], p[0:64, :], bkt[:, 0:1]
                            )
                            rot = rtmp.tile([64, SCQ], BF16, tag="rot",
                                            name=f"krot_{sc}")
                            nc.scalar.copy(rot[0:32, :], kb_[32:64, :])
                            nc.scalar.copy(rot[32:64, :], kb_[0:32, :])
                            nc.vector.tensor_mul(
                                kb_[:, :], kb_[:, :], cost[:, c0 : c0 + SCQ]
                            )
                            nc.vector.tensor_mul(
                                rot[:, :], rot[:, :], sint[:, c0 : c0 + SCQ]
                            )
                            nc.vector.tensor_add(
                                kh[:, c0 : c0 + SCQ], kb_[:, :], rot[:, :]
                            )
                            nc.vector.tensor_scalar_add(
                                vT[:, c0 : c0 + SCQ], p[64:128, :], bvt[:, 0:1]
                            )

            # ---------------- v transpose + ones column ----------------
            with tc.tile_pool(name="vtp", bufs=2, space="PSUM") as vtp:
                for kb in range(KBN):
                    pv = vtp.tile([128, D], BF16, tag="pv", name=f"pv{kb}")
                    nc.tensor.transpose(
                        pv[:, :], vT[:, kb * 128 : (kb + 1) * 128], ident[0:64, 0:64]
                    )
                    nc.vector.tensor_copy(vaug[:, kb * VA : kb * VA + 64], pv[:, :])
                    for oc in range(64, VA):
                        nc.vector.tensor_copy(
                            vaug[:, kb * VA + oc : kb * VA + oc + 1], ones[:, :]
                        )

            # ---------------- attention + wo + chunked ReduceScatter ----------------
            with (
                tc.tile_pool(name="oTp", bufs=1) as oTp,
                tc.tile_pool(name="woTp", bufs=1) as woTp,
                tc.tile_pool(name="esp", bufs=14) as esp,
                tc.tile_pool(name="scps", bufs=2, space="PSUM") as scps,
                tc.tile_pool(name="pops", bufs=1, space="PSUM") as pops,
                tc.tile_pool(name="wops", bufs=2, space="PSUM") as wops,
            ):
                oTt = oTp.tile([128, JBN * S], BF16, tag="oT")
                woTt = woTp.tile([128, JBN * HID], BF16, tag="woT")
                nc.sync.dma_start(
                    out=woTt[:, :].rearrange("p (jb m) -> p jb m", jb=JBN),
                    in_=woT[:, :].rearrange("(jb p) m -> p jb m", p=128),
                )

                deferred_out = []

                def emit_wo_block(qc, mb, piece_state, defer_out=False):
                    # One wo output-row block: 4 accumulating matmuls over jb,
                    # bias-add eviction (vector), DMA to yT_part, and the RS
                    # trigger + out-DMA when this mb completes a row piece.
                    q0 = qc * QC
                    rows = 128 if mb < CBF else 64
                    pw = wops.tile([128, QC], F32, tag="wo",
                                   name=f"pw_{qc}_{mb}")
                    for jb in range(JBN):
                        nc.tensor.matmul(
                            pw[0:rows, :],
                            woTt[:, jb * HID + mb * 128 : jb * HID + mb * 128 + rows],
                            oTt[:, jb * S + q0 : jb * S + q0 + QC],
                            start=(jb == 0),
                            stop=(jb == JBN - 1),
                        )
                    yt = ytp.tile([128, QC], BF16, tag="ytb",
                                  name=f"yt_{qc}_{mb}")
                    # qc0's burst runs after the last exps: scalar is idle
                    # there while vector still drains attention, so its
                    # evictions all go to scalar
                    if mb % 2 == 0 and qc != 0:
                        nc.vector.tensor_scalar_add(
                            yt[0:rows, :], pw[0:rows, :],
                            wob8t[0:rows, mb : mb + 1],
                        )
                    else:
                        nc.scalar.activation(
                            yt[0:rows, :], pw[0:rows, :],
                            mybir.ActivationFunctionType.Identity,
                            bias=wob8t[0:rows, mb : mb + 1],
                        )
                    nc.sync.dma_start(
                        out=yT_part[qc][mb * 128 : mb * 128 + rows, :],
                        in_=yt[0:rows, :],
                    )
                    piece = piece_state[0]
                    if (piece < len(RS_PIECES[qc])
                            and mb == RS_PIECES[qc][piece][0]):
                        _, r0, r1 = RS_PIECES[qc][piece]
                        nc.gpsimd.collective_compute(
                            "ReduceScatter",
                            mybir.AluOpType.add,
                            replica_groups=[list(range(CORES))],
                            ins=[yT_part[qc][r0:r1, :].opt()],
                            outs=[yT_red[(qc, piece)].ap().opt()],
                        )
                        o0 = r0 // CORES
                        nrows = (r1 - r0) // CORES
                        # issue from gpsimd: this DMA waits on the collective,
                        # and must not block the sync queue's evictions. For
                        # the last chunk, defer it past the remaining RS
                        # triggers so they launch back-to-back.
                        args = (out[o0 : o0 + nrows, q0 : q0 + QC],
                                yT_red[(qc, piece)].ap())
                        if defer_out:
                            deferred_out.append(args)
                        else:
                            nc.gpsimd.dma_start(out=args[0], in_=args[1])
                        piece_state[0] += 1

                def emit_score_pair(qc, h, pk):
                    # Two k-blocks into one 2-bank PSUM tile, one exp over
                    # [128, 1024] (halves the scalar engine's fixed cost).
                    # Diagonal k-blocks compute full width; the sub-diagonal
                    # garbage is never read (AV skips s < j) and only the
                    # diagonal 128-block needs the triangular mask.
                    q0 = qc * QC
                    qb0 = q0 // 128
                    ps_s = scps.tile([128, 2 * QC], F32, tag="scores",
                                     name=f"ps_{qc}_{h}_{pk}")
                    for half in range(2):
                        kb = 2 * pk + half
                        j = kb - qb0
                        w0 = 128 * j if j > 0 else 0
                        nc.tensor.matmul(
                            ps_s[:, half * QC + w0 : (half + 1) * QC],
                            kh[:, kb * 128 : (kb + 1) * 128],
                            qq[:, h * S + q0 + w0 : h * S + q0 + QC],
                            start=True,
                            stop=True,
                        )
                    es = esp.tile([128, 2 * QC], BF16, tag="es",
                                  name=f"es_{qc}_{h}_{pk}")
                    nc.scalar.activation(es[:, :], ps_s[:, :], Exp, scale=0.125)
                    for half in range(2):
                        kb = 2 * pk + half
                        j = kb - qb0
                        if j >= 0:
                            dc = half * QC + 128 * j
                            # gpsimd is collective-free until the first RS
                            # trigger (mid-attn of the second chunk), so the
                            # first chunk's even masks can run there
                            if qc == QC_ORDER[0] and kb % 2 == 0:
                                nc.gpsimd.affine_select(
                                    out=es[:, dc : dc + 128],
                                    in_=es[:, dc : dc + 128],
                                    compare_op=mybir.AluOpType.is_ge,
                                    fill=0.0,
                                    base=0,
                                    pattern=[[1, 128]],
                                    channel_multiplier=-1,
                                )
                            else:
                                nc.vector.tensor_mul(
                                    es[:, dc : dc + 128],
                                    es[:, dc : dc + 128],
                                    tri[:, :],
                                )
                    return es

                def emit_av_wave(qc, h, es_pairs, wave):
                    # AV accumulation for q sub-blocks (2*wave, 2*wave+1) plus
                    # the vector normalize chain; the pt transposes are
                    # deferred (emit_pts) so the PE isn't parked behind the
                    # vector chain.
                    qb0 = qc * QC // 128
                    po = {}
                    for s in (2 * wave, 2 * wave + 1):
                        po[s] = pops.tile([128, VA], F32, tag=f"po{s % 2}",
                                          name=f"po{s}_{qc}_{h}")
                        for kb in range(qb0 + s + 1):
                            nc.tensor.matmul(
                                po[s][:, :],
                                es_pairs[kb // 2][:, (kb % 2) * QC + s * 128 :
                                                  (kb % 2) * QC + s * 128 + 128],
                                vaug[:, kb * VA : (kb + 1) * VA],
                                start=(kb == 0),
                                stop=(kb == qb0 + s),
                            )
                    o_ns = {}
                    for s in (2 * wave, 2 * wave + 1):
                        denom = small.tile([128, 1], F32, tag="denom",
                                           name=f"dn_{qc}_{h}_{s}")
                        nc.vector.tensor_add(
                            denom[:, :], po[s][:, D : D + 1], esk[:, h : h + 1]
                        )
                        recip = small.tile([128, 1], F32, tag="recip",
                                           name=f"rc_{qc}_{h}_{s}")
                        nc.vector.reciprocal(recip[:, :], denom[:, :])
                        o_n = small.tile([128, D], BF16, tag="o_n",
                                         name=f"on_{qc}_{h}_{s}")
                        nc.vector.tensor_scalar_mul(
                            o_n[:, :], po[s][:, 0:D], recip[:, :]
                        )
                        o_ns[s] = o_n
                    return o_ns

                def emit_pts(qc, h, o_ns):
                    # Transpose normalized outputs into oTt; pt tiles reuse
                    # the po banks (tag-shared) to stay inside the 8-bank
                    # PSUM budget.
                    q0 = qc * QC
                    for s, o_n in o_ns.items():
                        pt = pops.tile([64, 128], BF16, tag=f"po{s % 2}",
                                       name=f"pt_{qc}_{h}_{s}")
                        nc.tensor.transpose(pt[:, :], o_n[:, :], ident[:, :])
                        jb, ro = h // 2, (h % 2) * 64
                        dst = oTt[
                            ro : ro + 64,
                            jb * S + q0 + s * 128 : jb * S + q0 + (s + 1) * 128,
                        ]
                        if s % 2 == 0:
                            nc.vector.tensor_copy(dst, pt[:, :])
                        else:
                            nc.scalar.copy(dst, pt[:, :])

                # Software pipeline: head h's score pairs are spread between
                # head h-1's AV waves (so exps drain in the shadow of AV/wo
                # work instead of stalling the score PSUM rotation), and the
                # previous chunk's wo blocks fill remaining PE slack. The PE
                # stays continuously busy, which also keeps its DVFS p-state
                # at full clock.
                WO_PER_SLOT = 6
                pending = []

                def fill_wo(n):
                    for _ in range(n):
                        if pending:
                            pending.pop(0)()

                def slot(qc, h, prev_es):
                    # Emit one pipeline slot: all score pairs for head h, then
                    # the AV waves + normalize + transposes of head h-1
                    # (prev_es), then wo filler. h == HL drains only.
                    npairs = (qc * QC // 128 + 4) // 2
                    es_pairs = []
                    if h < HL:
                        for pk in range(npairs):
                            es_pairs.append(emit_score_pair(qc, h, pk))
                    if prev_es:
                        o_a = emit_av_wave(qc, h - 1, prev_es, 0)
                        emit_pts(qc, h - 1, o_a)
                        o_b = emit_av_wave(qc, h - 1, prev_es, 1)
                        emit_pts(qc, h - 1, o_b)
                    fill_wo(WO_PER_SLOT)
                    return es_pairs

                for qc in QC_ORDER:
                    prev = None
                    for h in range(HL + 1):
                        prev = slot(qc, h, prev)
                    while pending:
                        pending.pop(0)()
                    ps_ = [0]
                    if qc != 0:
                        pending = [
                            (lambda qc=qc, mb=mb, ps_=ps_:
                             emit_wo_block(qc, mb, ps_))
                            for mb in range(MBN)
                        ]
                    else:
                        for mb in range(MBN):
                            emit_wo_block(qc, mb, ps_)

    nc.finalize()
    return nc


def make_in_maps(x, rope_cache, wq_w, wq_b, wk_w, wk_b, wv_w, wv_b, wo_w, wo_b, sinks):
    import ml_dtypes

    BF = ml_dtypes.bfloat16
    xT = np.ascontiguousarray(x[0].T).astype(BF)  # [2880, 1536]
    cosT = np.ascontiguousarray(rope_cache[:, :D].T, dtype=np.float32)
    sinT = np.ascontiguousarray(rope_cache[:, D:].T, dtype=np.float32)
    sinTs = sinT.copy()
    sinTs[: D // 2] *= -1.0
    cosT = cosT.astype(BF)
    sinTs = sinTs.astype(BF)

    in_maps = []
    for i in range(CORES):
        wq = wq_w[512 * i : 512 * (i + 1)]
        wk = wk_w[64 * i : 64 * (i + 1)]
        wv = wv_w[64 * i : 64 * (i + 1)]
        wT = np.ascontiguousarray(np.concatenate([wq, wk, wv], axis=0).T).astype(BF)
        bq = np.ascontiguousarray(
            wq_b[512 * i : 512 * (i + 1)].reshape(HL, 64).T, np.float32
        )
        bk = wk_b[64 * i : 64 * (i + 1)].reshape(64, 1).astype(np.float32)
        bv = wv_b[64 * i : 64 * (i + 1)].reshape(64, 1).astype(np.float32)
        woT = np.ascontiguousarray(wo_w[:, 512 * i : 512 * (i + 1)].T).astype(BF)
        wob8 = np.zeros((128, MBN), np.float32)
        for mb in range(MBN):
            piece = wo_b[128 * mb : 128 * (mb + 1)] / 8.0
            wob8[: len(piece), mb] = piece
        es = np.exp(sinks[HL * i : HL * (i + 1)]).astype(np.float32)
        esink = np.repeat(es[None, :], 128, axis=0).astype(np.float32)
        in_maps.append(
            {
                "xT": xT,
                "wT": wT,
                "bq": bq,
                "bk": bk,
                "bv": bv,
                "cosT": cosT,
                "sinTs": sinTs,
                "woT": woT,
                "wob8": np.ascontiguousarray(wob8),
                "esink": esink,
            }
        )
    return in_maps


_CACHE = {}


def kernel(**inputs):
    _install_hooks()
    from concourse import bass_utils

    trace = bool(int(os.environ.get("BASS_KERNEL_TRACE", "0")))
    if "nc" not in _CACHE:
        _CACHE["nc"] = build_graph()
    nc = _CACHE["nc"]

    in_maps = make_in_maps(**{k: np.asarray(v) for k, v in inputs.items()})
    res = bass_utils.run_bass_kernel_spmd(
        nc, in_maps, core_ids=list(range(CORES)), trace=trace
    )
    _EXEC_TIME_NS[0] = res.exec_time_ns

    # Assemble: piece (qc, p) covering yT rows [r0, r1) sits at rows
    # [r0/8, r1/8) of core i's 360-row shard and holds global yT rows
    # [r0 + i*(r1-r0)/8, ...) for q columns [qc*QC, qc*QC+QC).
    RS_PIECES = {2: [(0, 1408), (1408, HID)], 1: [(0, HID)],
                 0: [(0, 1024), (1024, HID)]}
    y = np.empty((S, HID), np.float32)
    for i in range(CORES):
        o = res.results[i]["out"].astype(np.float32)
        for qc in range(NQC):
            for r0, r1 in RS_PIECES[qc]:
                rows = (r1 - r0) // CORES
                o0 = r0 // CORES
                y[qc * QC : (qc + 1) * QC,
                  r0 + i * rows : r0 + (i + 1) * rows] = (
                    o[o0 : o0 + rows, qc * QC : (qc + 1) * QC].T
                )
    return y.reshape(1, S, HID)


def last_exec_time_ns():
    return _EXEC_TIME_NS[0]

